# revision 7
# baseline (speedup 1.0000x reference)
"""GPT-2 small forward pass on 8 TRN2 NeuronCores (Bass/Tile).

Sharding: 8 cores = 4 batch elements x 2 sequence halves (512 tokens each).
Each core runs the full 12-layer trunk on its 512 tokens with replicated
weights; the only cross-core traffic is a per-layer 2-core AllGather of K/V
within each batch pair. Attention uses a transposed-score layout (keys on
partitions, queries on free dim) so the softmax denominator falls out of a
ones-augmented V matmul; causal masking is a multiplicative {0,1} bf16 mask
passed as per-core input data. All matmuls bf16 with fp32 PSUM accumulation;
residual stream and layernorm statistics in fp32.

Host side: embedding gather (W_E[tokens] + W_pos), weight repacking/bf16
cast, final unshard + b_U add.

Self-contained: only numpy/ml_dtypes/concourse imports; all shapes hardcoded.
"""

import os
import numpy as np
import ml_dtypes

import concourse.bass as bass
import concourse.mybir as mybir
import concourse.tile as tile
from concourse import bacc
from concourse import bass_utils

F32 = mybir.dt.float32
BF16 = mybir.dt.bfloat16
AF = mybir.ActivationFunctionType
ALU = mybir.AluOpType

# model dims
B, S, D, H, DH, DM, L, VOCAB = 4, 1024, 768, 12, 64, 3072, 12, 50257
T = 512              # tokens per core
NCORES = 8
DT = D // 128        # 6  d-tiles
MT = DM // 128       # 24 m-tiles of d_mlp
VS = (VOCAB + 511) // 512   # 99 vocab slices
VPAD = VS * 512      # 50688
EPS = 1e-5
PAIRS = [[0, 1], [2, 3], [4, 5], [6, 7]]

# bias-pack columns inside the per-layer [128, 840] f32 tensor
BC_QKVB = 0     # 12 cols: Q then K feature-tile biases
BC_BO = 12      # 6
BC_BIN = 18     # 24
BC_BOUT = 42    # 6
BC_L1W = 48     # 6
BC_L1B = 54     # 6
BC_L2W = 60     # 6
BC_L2B = 66     # 6
BC_BV = 72      # 768 (host-replicated across partitions)
BCOLS = 840

_BUILD_CACHE = {}


def _layernorm(nc, sb, rows, ps512, lntmp, x_sb, bias_sb, wcol, bcol, out_bf,
               ones_bf, ones_f32, eps_sb, n_feat=D):
    """out_bf[128, DT, T] (bf16) = LN(x_sb[128, DT, T] f32) * w + b.

    Feature axis is (partition, d-tile): cross-partition sums via ones-matmul.
    wcol/bcol are column offsets into bias_sb [128, BCOLS] (per-partition,
    per-d-tile scalars).
    """
    xb = sb.tile([128, DT, T], BF16, tag="lnxb", bufs=1)
    nc.scalar.copy(xb[:, :, :], x_sb[:, :, :])
    xsq = sb.tile([128, DT, T], BF16, tag="lnxsq", bufs=1)
    nc.vector.tensor_mul(xsq[:, :, :], xb[:, :, :], xb[:, :, :])

    s1 = ps512.tile([1, T], F32, tag="ps512", name="ln_s1")
    s2 = ps512.tile([1, T], F32, tag="ps512", name="ln_s2")
    for i in range(DT):
        nc.tensor.matmul(s1[:, :], ones_bf[:, 0:1], xb[:, i, :],
                         start=(i == 0), stop=(i == DT - 1))
    for i in range(DT):
        nc.tensor.matmul(s2[:, :], ones_bf[:, 0:1], xsq[:, i, :],
                         start=(i == 0), stop=(i == DT - 1))

    row_mean = rows.tile([1, T], F32, tag="rows", name="ln_mean")
    nc.vector.tensor_scalar_mul(row_mean[:, :], s1[:, :], 1.0 / n_feat)
    row_m2 = rows.tile([1, T], F32, tag="rows", name="ln_m2")
    nc.vector.tensor_mul(row_m2[:, :], s1[:, :], row_mean[:, :])
    row_var = rows.tile([1, T], F32, tag="rows", name="ln_var")
    nc.vector.tensor_sub(row_var[:, :], s2[:, :], row_m2[:, :])
    # unbiased variance (ddof=1): var = (sum_x2 - sum_x*mean) / (n-1)
    row_std = rows.tile([1, T], F32, tag="rows", name="ln_std")
    nc.scalar.activation(row_std[:, :], row_var[:, :], AF.Sqrt,
                         bias=eps_sb[0:1, 0:1], scale=1.0 / (n_feat - 1))
    row_inv = rows.tile([1, T], F32, tag="rows", name="ln_inv")
    nc.vector.reciprocal(row_inv[:, :], row_std[:, :])

    mb = ps512.tile([128, T], F32, tag="ps512", name="ln_mb")
    nc.tensor.matmul(mb[:, :], ones_f32[0:1, 0:128], row_mean[:, :],
                     start=True, stop=True)
    ib = ps512.tile([128, T], F32, tag="ps512", name="ln_ib")
    nc.tensor.matmul(ib[:, :], ones_f32[0:1, 0:128], row_inv[:, :],
                     start=True, stop=True)

    for i in range(DT):
        t0 = lntmp.tile([128, T], F32, tag="lntmp", name=f"ln_t{i}")
        nc.vector.tensor_sub(t0[:, :], x_sb[:, i, :], mb[:, :])
        u0 = lntmp.tile([128, T], F32, tag="lntmp", name=f"ln_u{i}")
        nc.vector.tensor_mul(u0[:, :], t0[:, :], ib[:, :])
        nc.vector.tensor_scalar(out_bf[:, i, :], u0[:, :],
                                bias_sb[:, wcol + i:wcol + i + 1],
                                bias_sb[:, bcol + i:bcol + i + 1],
                                ALU.mult, ALU.add)


def build(n_layers=L, n_vslices=VS):
    """Build + compile the SPMD kernel. Returns the Bacc object."""
    key = (n_layers, n_vslices)
    if key in _BUILD_CACHE:
        return _BUILD_CACHE[key]

    nc = bacc.Bacc("TRN2", target_bir_lowering=False, debug=False,
                   enable_asserts=False, num_devices=NCORES)

    # ---- kernel I/O (per-core shards; all cores same shapes) ----
    x0_d = nc.dram_tensor("x0", [128, DT, T], F32, kind="ExternalInput")
    wqkv_d = nc.dram_tensor("wqkv", [n_layers, 128, DT, 3 * D], BF16,
                            kind="ExternalInput")
    wo_d = nc.dram_tensor("wo", [n_layers, 128, DT, D], BF16,
                          kind="ExternalInput")
    win_d = nc.dram_tensor("win", [n_layers, 128, DT, DM], BF16,
                           kind="ExternalInput")
    wout_d = nc.dram_tensor("wout", [n_layers, DT, 128, MT, 128], BF16,
                            kind="ExternalInput")
    wu_d = nc.dram_tensor("wu", [n_vslices, 128, DT, 512], BF16,
                          kind="ExternalInput")
    bias_d = nc.dram_tensor("biases", [n_layers, 128, BCOLS], F32,
                            kind="ExternalInput")
    lnf_d = nc.dram_tensor("lnf", [128, 2 * DT], F32, kind="ExternalInput")
    mask_d = nc.dram_tensor("mask", [128, 8, T], BF16, kind="ExternalInput")
    out_d = nc.dram_tensor("out", [4, 128, n_vslices, 512], BF16,
                           kind="ExternalOutput")

    with tile.TileContext(nc) as tc:
        with tc.tile_pool(name="sb", bufs=1) as sb, \
             tc.tile_pool(name="rows", bufs=6) as rows, \
             tc.tile_pool(name="lntmp", bufs=2) as lntmp, \
             tc.tile_pool(name="ps512", bufs=4, space="PSUM") as ps512, \
             tc.tile_pool(name="pso", bufs=2, space="PSUM") as pso, \
             tc.tile_pool(name="dram", bufs=2, space="DRAM") as dram:

            # ---- persistent tiles ----
            ones_f32 = sb.tile([128, 128], F32, tag="ones_f32")
            nc.vector.memset(ones_f32[:, :], 1.0)
            ones_bf = sb.tile([128, 1], BF16, tag="ones_bf")
            nc.vector.memset(ones_bf[:, :], 1.0)
            eps_sb = sb.tile([128, 1], F32, tag="eps")
            nc.vector.memset(eps_sb[:, :], EPS)

            xT = sb.tile([128, DT, T], F32, tag="xT")
            nc.sync.dma_start(xT[:, :, :], x0_d.ap())

            mask_sb = sb.tile([128, 8, T], BF16, tag="mask")
            nc.sync.dma_start(mask_sb[:, :, :], mask_d.ap())

            # K/V of both sequence halves, in global token order
            ktall = sb.tile([128, DT, 2 * T], BF16, tag="ktall")
            vall = sb.tile([128, 8, H, 65], BF16, tag="vall")
            nc.vector.memset(vall[:, :, :, 64:65], 1.0)  # denominator ones col

            for layer in range(n_layers):
                bias_sb = sb.tile([128, BCOLS], F32, tag="bias", bufs=2,
                                  name=f"bias_l{layer}")
                nc.sync.dma_start(bias_sb[:, :], bias_d[layer, :, :])

                # ---- LN1 ----
                h_bf = sb.tile([128, DT, T], BF16, tag="h", bufs=2,
                               name=f"h1_l{layer}")
                _layernorm(nc, sb, rows, ps512, lntmp, xT, bias_sb,
                           BC_L1W, BC_L1B, h_bf, ones_bf, ones_f32, eps_sb)

                # ---- K projection (feature-major KT) ----
                kt_own = sb.tile([128, DT, T], BF16, tag="kta", bufs=2,
                                 name=f"ktown_l{layer}")
                for m in range(6, 12):
                    wtile = sb.tile([128, DT, 128], BF16, tag="wqk", bufs=3,
                                    name=f"wk_l{layer}_m{m}")
                    nc.sync.dma_start(wtile[:, :, :],
                                      wqkv_d[layer, :, :, 128 * m:128 * m + 128])
                    psq = ps512.tile([128, T], F32, tag="ps512",
                                     name=f"psk_l{layer}_m{m}")
                    for i in range(DT):
                        nc.tensor.matmul(psq[:, :], wtile[:, i, :], h_bf[:, i, :],
                                         start=(i == 0), stop=(i == DT - 1))
                    nc.vector.tensor_scalar_add(
                        kt_own[:, m - 6, :], psq[:, :],
                        bias_sb[:, BC_QKVB + m:BC_QKVB + m + 1])

                # ---- V projection (token-major, lhsT = h tiles) ----
                v_own = sb.tile([128, 4, H, 64], BF16, tag="vown", bufs=1,
                                name=f"vown_l{layer}")
                for half in range(2):
                    wv = sb.tile([128, DT, 384], BF16, tag="wv", bufs=2,
                                 name=f"wv_l{layer}_{half}")
                    nc.sync.dma_start(
                        wv[:, :, :],
                        wqkv_d[layer, :, :, 2 * D + 384 * half:2 * D + 384 * (half + 1)])
                    for tt in range(4):
                        psv = ps512.tile([128, 384], F32, tag="ps512",
                                         name=f"psv_l{layer}_{half}_{tt}")
                        for i in range(DT):
                            nc.tensor.matmul(psv[:, :],
                                             h_bf[:, i, 128 * tt:128 * tt + 128],
                                             wv[:, i, :],
                                             start=(i == 0), stop=(i == DT - 1))
                        nc.vector.tensor_add(
                            v_own[:, tt, 6 * half:6 * half + 6, :], psv[:, :],
                            bias_sb[:, BC_BV + 384 * half:BC_BV + 384 * (half + 1)])

                # ---- pair exchange of K/V ----
                bounce_in = dram.tile([128, 2 * DT * T], BF16, tag="cin",
                                      name=f"cin_l{layer}")
                bounce_out = dram.tile([256, 2 * DT * T], BF16, tag="cout",
                                       name=f"cout_l{layer}")
                nc.sync.dma_start(bounce_in[:, 0:DT * T], kt_own[:, :, :])
                nc.sync.dma_start(bounce_in[:, DT * T:2 * DT * T],
                                  v_own[:, :, :, :])
                nc.gpsimd.collective_compute(
                    "AllGather", ALU.bypass, replica_groups=PAIRS,
                    ins=[bounce_in[:, :].opt()], outs=[bounce_out[:, :].opt()])
                bo_r = bounce_out[:, :].rearrange(
                    "(c p) (i t) -> c p i t", c=2, i=2 * DT)
                for c in range(2):
                    nc.sync.dma_start(ktall[:, :, T * c:T * (c + 1)],
                                      bo_r[c, :, 0:DT, :])
                    nc.sync.dma_start(
                        vall[:, 4 * c:4 * (c + 1), :, 0:64],
                        bounce_out[:, :].rearrange(
                            "(c p) (tt h e) -> c p tt h e",
                            c=2, tt=2 * 4, h=H)[c, :, 4:8, :, :])

                # ---- Q projection (overlaps the collective) ----
                qt = sb.tile([128, DT, T], BF16, tag="qt", bufs=1,
                             name=f"qt_l{layer}")
                for m in range(6):
                    wtile = sb.tile([128, DT, 128], BF16, tag="wqk", bufs=3,
                                    name=f"wq_l{layer}_m{m}")
                    nc.sync.dma_start(wtile[:, :, :],
                                      wqkv_d[layer, :, :, 128 * m:128 * m + 128])
                    psq = ps512.tile([128, T], F32, tag="ps512",
                                     name=f"psq_l{layer}_m{m}")
                    for i in range(DT):
                        nc.tensor.matmul(psq[:, :], wtile[:, i, :], h_bf[:, i, :],
                                         start=(i == 0), stop=(i == DT - 1))
                    # (Q + b) / sqrt(DH)
                    nc.vector.tensor_scalar(
                        qt[:, m, :], psq[:, :],
                        bias_sb[:, BC_QKVB + m:BC_QKVB + m + 1], 0.125,
                        ALU.add, ALU.mult)

                # ---- attention, head-pair software pipeline ----
                attnT = sb.tile([128, DT, T], BF16, tag="kta", bufs=2,
                                name=f"attnT_l{layer}")
                prev = None
                for hd in range(H + 1):
                    if hd < H:
                        r, hp = hd % 2, hd // 2
                        esm_list = []
                        for kt in range(8):
                            sps = ps512.tile([128, T], F32, tag="ps512",
                                             name=f"s_l{layer}_h{hd}_k{kt}")
                            nc.tensor.matmul(
                                sps[:, :],
                                ktall[64 * r:64 * r + 64, hp,
                                      128 * kt:128 * kt + 128],
                                qt[64 * r:64 * r + 64, hp, :],
                                start=True, stop=True)
                            es = sb.tile([128, T], BF16, tag="es", bufs=2,
                                         name=f"es_l{layer}_h{hd}_k{kt}")
                            nc.scalar.activation(es[:, :], sps[:, :], AF.Exp)
                            esm = sb.tile([128, T], BF16, tag=f"esm{hd % 2}",
                                          bufs=8,
                                          name=f"esm_l{layer}_h{hd}_k{kt}")
                            nc.vector.tensor_mul(esm[:, :], es[:, :],
                                                 mask_sb[:, kt, :])
                            esm_list.append(esm)
                    if prev is not None:
                        phd, plist = prev
                        pr, php = phd % 2, phd // 2
                        po = pso.tile([65, T], F32, tag="pso",
                                      name=f"po_l{layer}_h{phd}")
                        for kt in range(8):
                            nc.tensor.matmul(po[:, :], vall[:, kt, phd, :],
                                             plist[kt][:, :],
                                             start=(kt == 0), stop=(kt == 7))
                        rinv = rows.tile([1, T], F32, tag="rows",
                                         name=f"ainv_l{layer}_h{phd}")
                        nc.vector.reciprocal(rinv[:, :], po[64:65, :])
                        ibp = ps512.tile([64, T], F32, tag="ps512",
                                         name=f"aib_l{layer}_h{phd}")
                        nc.tensor.matmul(ibp[:, :], ones_f32[0:1, 0:64],
                                         rinv[:, :], start=True, stop=True)
                        ibs = sb.tile([64, T], F32, tag="ibs", bufs=1,
                                      name=f"aibs_l{layer}_h{phd}")
                        nc.scalar.copy(ibs[:, :], ibp[:, :])
                        nc.vector.tensor_mul(
                            attnT[64 * pr:64 * pr + 64, php, :],
                            po[0:64, :], ibs[:, :])
                    prev = (hd, esm_list) if hd < H else None

                # ---- attn output projection + residual ----
                for i in range(DT):
                    wtile = sb.tile([128, DT, 128], BF16, tag="wqk", bufs=3,
                                    name=f"wo_l{layer}_i{i}")
                    nc.sync.dma_start(wtile[:, :, :],
                                      wo_d[layer, :, :, 128 * i:128 * i + 128])
                    pao = ps512.tile([128, T], F32, tag="ps512",
                                     name=f"pao_l{layer}_i{i}")
                    for j in range(DT):
                        nc.tensor.matmul(pao[:, :], wtile[:, j, :],
                                         attnT[:, j, :],
                                         start=(j == 0), stop=(j == DT - 1))
                    # x = x + attn_out + b_O
                    nc.vector.scalar_tensor_tensor(
                        xT[:, i, :], pao[:, :],
                        bias_sb[:, BC_BO + i:BC_BO + i + 1], xT[:, i, :],
                        ALU.add, ALU.add)

                # ---- LN2 ----
                h2 = sb.tile([128, DT, T], BF16, tag="h", bufs=2,
                             name=f"h2_l{layer}")
                _layernorm(nc, sb, rows, ps512, lntmp, xT, bias_sb,
                           BC_L2W, BC_L2B, h2, ones_bf, ones_f32, eps_sb)

                # ---- MLP in + gelu ----
                gT = sb.tile([128, MT, T], BF16, tag="gT",
                             name=f"gT_l{layer}")
                for j in range(MT):
                    wtile = sb.tile([128, DT, 128], BF16, tag="wqk", bufs=3,
                                    name=f"wi_l{layer}_j{j}")
                    nc.sync.dma_start(wtile[:, :, :],
                                      win_d[layer, :, :, 128 * j:128 * j + 128])
                    pg = ps512.tile([128, T], F32, tag="ps512",
                                    name=f"pg_l{layer}_j{j}")
                    for i in range(DT):
                        nc.tensor.matmul(pg[:, :], wtile[:, i, :], h2[:, i, :],
                                         start=(i == 0), stop=(i == DT - 1))
                    nc.scalar.activation(gT[:, j, :], pg[:, :],
                                         AF.Gelu_apprx_tanh,
                                         bias=bias_sb[:, BC_BIN + j:BC_BIN + j + 1])

                # ---- MLP out + residual ----
                for i in range(DT):
                    wtile = sb.tile([128, MT, 128], BF16, tag="wout", bufs=2,
                                    name=f"wo2_l{layer}_i{i}")
                    nc.sync.dma_start(wtile[:, :, :], wout_d[layer, i, :, :, :])
                    pm = ps512.tile([128, T], F32, tag="ps512",
                                    name=f"pm_l{layer}_i{i}")
                    for j in range(MT):
                        nc.tensor.matmul(pm[:, :], wtile[:, j, :], gT[:, j, :],
                                         start=(j == 0), stop=(j == MT - 1))
                    nc.vector.scalar_tensor_tensor(
                        xT[:, i, :], pm[:, :],
                        bias_sb[:, BC_BOUT + i:BC_BOUT + i + 1], xT[:, i, :],
                        ALU.add, ALU.add)

            # ---- final LN ----
            lnf_sb = sb.tile([128, 2 * DT], F32, tag="lnf")
            nc.sync.dma_start(lnf_sb[:, :], lnf_d.ap())
            xf = sb.tile([128, DT, T], BF16, tag="h", bufs=2, name="xf")
            _layernorm(nc, sb, rows, ps512, lntmp, xT, lnf_sb, 0, DT, xf,
                       ones_bf, ones_f32, eps_sb)

            # ---- unembedding: logits[t, v] for all padded vocab slices ----
            for s in range(n_vslices):
                wu = sb.tile([128, DT, 512], BF16, tag="wu", bufs=3,
                             name=f"wu_s{s}")
                nc.sync.dma_start(wu[:, :, :], wu_d[s, :, :, :])
                for tt in range(4):
                    pu = ps512.tile([128, 512], F32, tag="ps512",
                                    name=f"pu_s{s}_t{tt}")
                    for i in range(DT):
                        nc.tensor.matmul(pu[:, :],
                                         xf[:, i, 128 * tt:128 * tt + 128],
                                         wu[:, i, :],
                                         start=(i == 0), stop=(i == DT - 1))
                    ou = sb.tile([128, 512], BF16, tag="ou", bufs=2,
                                 name=f"ou_s{s}_t{tt}")
                    if tt % 2 == 0:
                        nc.vector.tensor_copy(ou[:, :], pu[:, :])
                    else:
                        nc.scalar.copy(ou[:, :], pu[:, :])
                    nc.sync.dma_start(out_d[tt, :, s, :], ou[:, :])

    nc.compile()
    _BUILD_CACHE[key] = nc
    return nc


def _to_bf16(x):
    return np.ascontiguousarray(x.astype(ml_dtypes.bfloat16))


def prep_in_maps(inputs, n_layers=L, n_vslices=VS):
    """Host-side sharding: returns list of 8 per-core input dicts."""
    f = lambda k: np.asarray(inputs[k], dtype=np.float32)
    tokens = np.asarray(inputs["tokens"])
    W_E, W_pos = f("W_E"), f("W_pos")
    x_full = W_E[tokens] + W_pos[None, :S, :]        # [4, 1024, 768] f32

    nl = n_layers
    # fused QKV weight, feature-major lhsT layout [L, 128, DT, 2304]
    wq = f("W_Q").transpose(0, 2, 1, 3).reshape(L, D, D)[:nl]
    wk = f("W_K").transpose(0, 2, 1, 3).reshape(L, D, D)[:nl]
    wv = f("W_V").transpose(0, 2, 1, 3).reshape(L, D, D)[:nl]
    wqkv = np.concatenate([wq, wk, wv], axis=2)       # [nl, 768, 2304]
    wqkv = _to_bf16(wqkv.reshape(nl, DT, 128, 3 * D).transpose(0, 2, 1, 3))

    wo = f("W_O").reshape(L, D, D)[:nl]               # rows e = h*64+eh
    wo = _to_bf16(wo.reshape(nl, DT, 128, D).transpose(0, 2, 1, 3))

    win = f("W_in")[:nl]                              # [nl, 768, 3072]
    win = _to_bf16(win.reshape(nl, DT, 128, DM).transpose(0, 2, 1, 3))

    wout = f("W_out")[:nl]                            # [nl, 3072, 768]
    wout = _to_bf16(wout.reshape(nl, MT, 128, DT, 128).transpose(0, 3, 2, 1, 4))

    wu_pad = np.zeros((D, VPAD), np.float32)
    wu_pad[:, :VOCAB] = f("W_U")
    wu = _to_bf16(wu_pad.reshape(DT, 128, VS, 512).transpose(2, 1, 0, 3))
    wu = np.ascontiguousarray(wu[:n_vslices])

    def percol(x, n):  # [nl, n*128] -> [nl, 128, n]
        return x.reshape(nl, n, 128).transpose(0, 2, 1)

    biases = np.zeros((nl, 128, BCOLS), np.float32)
    bq = f("b_Q").reshape(L, D)[:nl]
    bk = f("b_K").reshape(L, D)[:nl]
    biases[:, :, BC_QKVB:BC_QKVB + 12] = percol(
        np.concatenate([bq, bk], axis=1), 12)
    biases[:, :, BC_BO:BC_BO + DT] = percol(f("b_O")[:nl], DT)
    biases[:, :, BC_BIN:BC_BIN + MT] = percol(f("b_in")[:nl], MT)
    biases[:, :, BC_BOUT:BC_BOUT + DT] = percol(f("b_out")[:nl], DT)
    biases[:, :, BC_L1W:BC_L1W + DT] = percol(f("ln1_w")[:nl], DT)
    biases[:, :, BC_L1B:BC_L1B + DT] = percol(f("ln1_b")[:nl], DT)
    biases[:, :, BC_L2W:BC_L2W + DT] = percol(f("ln2_w")[:nl], DT)
    biases[:, :, BC_L2B:BC_L2B + DT] = percol(f("ln2_b")[:nl], DT)
    bv = f("b_V").reshape(L, D)[:nl]
    biases[:, :, BC_BV:BC_BV + D] = np.repeat(bv[:, None, :], 128, axis=1)

    lnf = np.zeros((128, 2 * DT), np.float32)
    lnf[:, 0:DT] = f("lnf_w").reshape(DT, 128).T
    lnf[:, DT:2 * DT] = f("lnf_b").reshape(DT, 128).T

    # per-parity causal mask: key(global)=128*kt+p  vs  query(global)=512*h+q
    kk = np.arange(128)[:, None, None]
    tt = np.arange(8)[None, :, None]
    qq = np.arange(T)[None, None, :]
    masks = []
    for h in range(2):
        m = (128 * tt + kk <= 512 * h + qq).astype(np.float32)
        masks.append(_to_bf16(m))

    in_maps = []
    for c in range(NCORES):
        b, h = c // 2, c % 2
        xh = x_full[b, T * h:T * (h + 1)]             # [512, 768]
        x0 = np.ascontiguousarray(
            xh.reshape(T, DT, 128).transpose(2, 1, 0)).astype(np.float32)
        in_maps.append({
            "x0": x0, "wqkv": wqkv, "wo": wo, "win": win, "wout": wout,
            "wu": wu, "biases": biases, "lnf": lnf, "mask": masks[h],
        })
    return in_maps


def assemble_output(results, inputs, n_vslices=VS):
    """results: list of 8 per-core out dicts -> full [4, 1024, VOCAB] f32."""
    vp = n_vslices * 512
    out = np.zeros((B, S, VOCAB), np.float32)
    for c in range(NCORES):
        b, h = c // 2, c % 2
        arr = np.asarray(results[c]["out"]).astype(np.float32)  # [4,128,vs,512]
        flat = arr.reshape(T, vp)[:, :min(vp, VOCAB)]
        out[b, T * h:T * h + T, :flat.shape[1]] = flat
    out += np.asarray(inputs["b_U"], dtype=np.float32)[None, None, :]
    return out


def install_trace_hook():
    """Register the axon NTFF profiling hook (missing from this image's
    antenv) so run_bass_kernel_spmd(trace=True) returns exec_time_ns."""
    import sys as _sys
    import types as _types
    import ctypes as _ctypes
    import contextlib as _contextlib
    if "antenv.axon_hooks" in _sys.modules:
        return

    def _make_hook():
        lib = _ctypes.CDLL("/opt/axon/libaxon_pjrt.so")
        if not hasattr(lib, "axon_start_nrt_profile"):
            return None
        lib.axon_start_nrt_profile.argtypes = [
            _ctypes.POINTER(_ctypes.c_int64), _ctypes.c_size_t]
        lib.axon_start_nrt_profile.restype = _ctypes.c_int64
        lib.axon_stop_nrt_profile.argtypes = [_ctypes.c_char_p]
        lib.axon_stop_nrt_profile.restype = _ctypes.c_int64

        @_contextlib.contextmanager
        def _hook(output_dir, device_ids):
            import jax
            jax.devices()
            if device_ids:
                ids = (_ctypes.c_int64 * len(device_ids))(*device_ids)
                rc = lib.axon_start_nrt_profile(ids, len(device_ids))
            else:
                rc = lib.axon_start_nrt_profile(None, 0)
            if rc != 0:
                raise RuntimeError(f"axon_start_nrt_profile rc={rc}")
            try:
                yield
            finally:
                lib.axon_stop_nrt_profile(str(output_dir).encode())
        return _hook

    mod = _types.ModuleType("antenv.axon_hooks")
    mod.get_axon_ntff_profile_hook = lambda: _make_hook()
    _sys.modules["antenv.axon_hooks"] = mod


def run(inputs, n_layers=L, n_vslices=VS, trace=False, tmpdir=None):
    """Build, run, and assemble. Returns (output, exec_time_ns)."""
    nc = build(n_layers, n_vslices)
    in_maps = prep_in_maps(inputs, n_layers, n_vslices)
    kwargs = {}
    if trace:
        install_trace_hook()
        tmpdir = tmpdir or "/tmp/bk_trace"
        os.makedirs(tmpdir, exist_ok=True)
        kwargs = dict(trace=True, tmpdir=tmpdir)
    res = bass_utils.run_bass_kernel_spmd(
        nc, in_maps, core_ids=list(range(NCORES)), **kwargs)
    out = assemble_output(res.results, inputs, n_vslices)
    return out, res.exec_time_ns


def kernel(**inputs):
    trace = bool(int(os.environ.get("BK_TRACE", "0")))
    out, t = run(inputs, trace=trace,
                 tmpdir=os.environ.get("BK_TRACE_DIR"))
    if trace:
        print(f"HW exec time: {t} ns")
    return out


# revision 10
# speedup vs baseline: 1.1518x; 1.1518x over previous
"""GPT-2 small forward pass on 8 TRN2 NeuronCores (Bass/Tile).

Sharding: 8 cores = 4 batch elements x 2 sequence halves (512 tokens each).
Each core runs the full 12-layer trunk on its 512 tokens with replicated
weights; the only cross-core traffic is a per-layer 2-core AllGather of K/V
within each batch pair. Attention uses a transposed-score layout (keys on
partitions, queries on free dim) so the softmax denominator falls out of a
ones-augmented V matmul; causal masking is a multiplicative {0,1} bf16 mask
passed as per-core input data. All matmuls bf16 with fp32 PSUM accumulation;
residual stream and layernorm statistics in fp32.

Host side: embedding gather (W_E[tokens] + W_pos), weight repacking/bf16
cast, final unshard + b_U add.

Self-contained: only numpy/ml_dtypes/concourse imports; all shapes hardcoded.
"""

import os
import numpy as np
import ml_dtypes

import concourse.bass as bass
import concourse.mybir as mybir
import concourse.tile as tile
from concourse import bacc
from concourse import bass_utils

F32 = mybir.dt.float32
BF16 = mybir.dt.bfloat16
AF = mybir.ActivationFunctionType
ALU = mybir.AluOpType

# model dims
B, S, D, H, DH, DM, L, VOCAB = 4, 1024, 768, 12, 64, 3072, 12, 50257
T = 512              # tokens per core
NCORES = 8
DT = D // 128        # 6  d-tiles
MT = DM // 128       # 24 m-tiles of d_mlp
VS = (VOCAB + 511) // 512   # 99 vocab slices
VPAD = VS * 512      # 50688
EPS = 1e-5
PAIRS = [[0, 1], [2, 3], [4, 5], [6, 7]]

# bias-pack columns inside the per-layer [128, 840] f32 tensor
BC_QKVB = 0     # 12 cols: Q then K feature-tile biases
BC_BO = 12      # 6
BC_BIN = 18     # 24
BC_BOUT = 42    # 6
BC_L1W = 48     # 6
BC_L1B = 54     # 6
BC_L2W = 60     # 6
BC_L2B = 66     # 6
BC_BV = 72      # 768 (host-replicated across partitions)
BCOLS = 840

_BUILD_CACHE = {}


def _layernorm(nc, sb, rows, ps512, lntmp, x_sb, bias_sb, wcol, bcol, out_bf,
               ones_bf, ones_f32, eps_sb, n_feat=D):
    """out_bf[128, DT, T] (bf16) = LN(x_sb[128, DT, T] f32) * w + b.

    Feature axis is (partition, d-tile): cross-partition sums via ones-matmul.
    wcol/bcol are column offsets into bias_sb [128, BCOLS] (per-partition,
    per-d-tile scalars).
    """
    xb = sb.tile([128, DT, T], BF16, tag="lnxb", bufs=1)
    nc.scalar.copy(xb[:, :, :], x_sb[:, :, :])
    xsq = sb.tile([128, DT, T], BF16, tag="lnxsq", bufs=1)
    nc.vector.tensor_mul(xsq[:, :, :], xb[:, :, :], xb[:, :, :])

    s1 = ps512.tile([1, T], F32, tag="ps512", name="ln_s1")
    s2 = ps512.tile([1, T], F32, tag="ps512", name="ln_s2")
    for i in range(DT):
        nc.tensor.matmul(s1[:, :], ones_bf[:, 0:1], xb[:, i, :],
                         start=(i == 0), stop=(i == DT - 1))
    for i in range(DT):
        nc.tensor.matmul(s2[:, :], ones_bf[:, 0:1], xsq[:, i, :],
                         start=(i == 0), stop=(i == DT - 1))

    row_mean = rows.tile([1, T], F32, tag="rows", name="ln_mean")
    nc.vector.tensor_scalar_mul(row_mean[:, :], s1[:, :], 1.0 / n_feat)
    row_m2 = rows.tile([1, T], F32, tag="rows", name="ln_m2")
    nc.vector.tensor_mul(row_m2[:, :], s1[:, :], row_mean[:, :])
    row_var = rows.tile([1, T], F32, tag="rows", name="ln_var")
    nc.vector.tensor_sub(row_var[:, :], s2[:, :], row_m2[:, :])
    # unbiased variance (ddof=1): var = (sum_x2 - sum_x*mean) / (n-1)
    row_std = rows.tile([1, T], F32, tag="rows", name="ln_std")
    nc.scalar.activation(row_std[:, :], row_var[:, :], AF.Sqrt,
                         bias=eps_sb[0:1, 0:1], scale=1.0 / (n_feat - 1))
    row_inv = rows.tile([1, T], F32, tag="rows", name="ln_inv")
    nc.vector.reciprocal(row_inv[:, :], row_std[:, :])

    mb = ps512.tile([128, T], F32, tag="ps512", name="ln_mb")
    nc.tensor.matmul(mb[:, :], ones_f32[0:1, 0:128], row_mean[:, :],
                     start=True, stop=True)
    ib = ps512.tile([128, T], F32, tag="ps512", name="ln_ib")
    nc.tensor.matmul(ib[:, :], ones_f32[0:1, 0:128], row_inv[:, :],
                     start=True, stop=True)

    for i in range(DT):
        t0 = lntmp.tile([128, T], F32, tag="lntmp", name=f"ln_t{i}")
        nc.vector.tensor_sub(t0[:, :], x_sb[:, i, :], mb[:, :])
        u0 = lntmp.tile([128, T], F32, tag="lntmp", name=f"ln_u{i}")
        nc.vector.tensor_mul(u0[:, :], t0[:, :], ib[:, :])
        nc.vector.tensor_scalar(out_bf[:, i, :], u0[:, :],
                                bias_sb[:, wcol + i:wcol + i + 1],
                                bias_sb[:, bcol + i:bcol + i + 1],
                                ALU.mult, ALU.add)


def build(n_layers=L, n_vslices=VS):
    """Build + compile the SPMD kernel. Returns the Bacc object."""
    key = (n_layers, n_vslices)
    if key in _BUILD_CACHE:
        return _BUILD_CACHE[key]

    nc = bacc.Bacc("TRN2", target_bir_lowering=False, debug=False,
                   enable_asserts=False, num_devices=NCORES)

    # ---- kernel I/O (per-core shards; all cores same shapes) ----
    x0_d = nc.dram_tensor("x0", [128, DT, T], F32, kind="ExternalInput")
    wqk_d = nc.dram_tensor("wqk", [n_layers, 12, 128, DT, 128], BF16,
                           kind="ExternalInput")
    wv_d = nc.dram_tensor("wv", [n_layers, 2, 128, DT, 384], BF16,
                          kind="ExternalInput")
    wo_d = nc.dram_tensor("wo", [n_layers, DT, 128, DT, 128], BF16,
                          kind="ExternalInput")
    win_d = nc.dram_tensor("win", [n_layers, MT, 128, DT, 128], BF16,
                           kind="ExternalInput")
    wout_d = nc.dram_tensor("wout", [n_layers, DT, 128, MT, 128], BF16,
                            kind="ExternalInput")
    wu_d = nc.dram_tensor("wu", [n_vslices, 128, DT, 512], BF16,
                          kind="ExternalInput")
    bias_d = nc.dram_tensor("biases", [n_layers, 128, BCOLS], F32,
                            kind="ExternalInput")
    lnf_d = nc.dram_tensor("lnf", [128, 2 * DT], F32, kind="ExternalInput")
    mask_d = nc.dram_tensor("mask", [128, 8, T], BF16, kind="ExternalInput")
    out_d = nc.dram_tensor("out", [4, 128, n_vslices, 512], BF16,
                           kind="ExternalOutput")

    with tile.TileContext(nc) as tc:
        with tc.tile_pool(name="sb", bufs=1) as sb, \
             tc.tile_pool(name="rows", bufs=6) as rows, \
             tc.tile_pool(name="lntmp", bufs=2) as lntmp, \
             tc.tile_pool(name="ps512", bufs=4, space="PSUM") as ps512, \
             tc.tile_pool(name="pso", bufs=2, space="PSUM") as pso, \
             tc.tile_pool(name="dram", bufs=2, space="DRAM") as dram:

            # ---- persistent tiles ----
            ones_f32 = sb.tile([128, 128], F32, tag="ones_f32")
            nc.vector.memset(ones_f32[:, :], 1.0)
            ones_bf = sb.tile([128, 1], BF16, tag="ones_bf")
            nc.vector.memset(ones_bf[:, :], 1.0)
            eps_sb = sb.tile([128, 1], F32, tag="eps")
            nc.vector.memset(eps_sb[:, :], EPS)

            xT = sb.tile([128, DT, T], F32, tag="xT")
            nc.sync.dma_start(xT[:, :, :], x0_d.ap())

            mask_sb = sb.tile([128, 8, T], BF16, tag="mask")
            nc.sync.dma_start(mask_sb[:, :, :], mask_d.ap())

            # K/V of both sequence halves, in global token order
            ktall = sb.tile([128, DT, 2 * T], BF16, tag="ktall")
            vall = sb.tile([128, 8, H, 65], BF16, tag="vall")
            v_own = sb.tile([128, 4, H, 65], BF16, tag="vown")
            nc.vector.memset(v_own[:, :, :, 64:65], 1.0)  # denominator ones col

            for layer in range(n_layers):
                bias_sb = sb.tile([128, BCOLS], F32, tag="bias", bufs=2,
                                  name=f"bias_l{layer}")
                nc.sync.dma_start(bias_sb[:, :], bias_d[layer, :, :])

                # ---- LN1 ----
                h_bf = sb.tile([128, DT, T], BF16, tag="h", bufs=2,
                               name=f"h1_l{layer}")
                _layernorm(nc, sb, rows, ps512, lntmp, xT, bias_sb,
                           BC_L1W, BC_L1B, h_bf, ones_bf, ones_f32, eps_sb)

                # ---- K projection (feature-major KT) ----
                kt_own = sb.tile([128, DT, T], BF16, tag="kta", bufs=2,
                                 name=f"ktown_l{layer}")
                for m in range(6, 12):
                    wtile = sb.tile([128, DT, 128], BF16, tag="wqk", bufs=3,
                                    name=f"wk_l{layer}_m{m}")
                    nc.sync.dma_start(wtile[:, :, :], wqk_d[layer, m, :, :, :])
                    psq = ps512.tile([128, T], F32, tag="ps512",
                                     name=f"psk_l{layer}_m{m}")
                    for i in range(DT):
                        nc.tensor.matmul(psq[:, :], wtile[:, i, :], h_bf[:, i, :],
                                         start=(i == 0), stop=(i == DT - 1))
                    nc.vector.tensor_scalar_add(
                        kt_own[:, m - 6, :], psq[:, :],
                        bias_sb[:, BC_QKVB + m:BC_QKVB + m + 1])

                # ---- V projection (token-major, lhsT = h tiles) ----
                for half in range(2):
                    wv = sb.tile([128, DT, 384], BF16, tag="wv", bufs=2,
                                 name=f"wv_l{layer}_{half}")
                    nc.sync.dma_start(wv[:, :, :], wv_d[layer, half, :, :, :])
                    for tt in range(4):
                        psv = ps512.tile([128, 384], F32, tag="ps512",
                                         name=f"psv_l{layer}_{half}_{tt}")
                        for i in range(DT):
                            nc.tensor.matmul(psv[:, :],
                                             h_bf[:, i, 128 * tt:128 * tt + 128],
                                             wv[:, i, :],
                                             start=(i == 0), stop=(i == DT - 1))
                        nc.vector.tensor_add(
                            v_own[:, tt, 6 * half:6 * half + 6, 0:64], psv[:, :],
                            bias_sb[:, BC_BV + 384 * half:BC_BV + 384 * (half + 1)])

                # ---- pair exchange of K/V ----
                KTN = DT * T
                VN = 4 * H * 65
                bounce_in = dram.tile([128, KTN + VN], BF16, tag="cin",
                                      name=f"cin_l{layer}")
                bounce_out = dram.tile([256, KTN + VN], BF16, tag="cout",
                                       name=f"cout_l{layer}")
                nc.sync.dma_start(bounce_in[:, 0:KTN], kt_own[:, :, :])
                nc.sync.dma_start(bounce_in[:, KTN:KTN + VN],
                                  v_own[:, :, :, :])
                nc.gpsimd.collective_compute(
                    "AllGather", ALU.bypass, replica_groups=PAIRS,
                    ins=[bounce_in[:, :].opt()], outs=[bounce_out[:, :].opt()])
                for c in range(2):
                    nc.sync.dma_start(
                        ktall[:, :, T * c:T * (c + 1)],
                        bounce_out[128 * c:128 * c + 128, 0:KTN].rearrange(
                            "p (i t) -> p i t", i=DT))
                    nc.sync.dma_start(
                        vall[:, 4 * c:4 * (c + 1), :, :],
                        bounce_out[128 * c:128 * c + 128, KTN:KTN + VN])

                # ---- Q projection (overlaps the collective) ----
                qt = sb.tile([128, DT, T], BF16, tag="qt", bufs=1,
                             name=f"qt_l{layer}")
                for m in range(6):
                    wtile = sb.tile([128, DT, 128], BF16, tag="wqk", bufs=3,
                                    name=f"wq_l{layer}_m{m}")
                    nc.sync.dma_start(wtile[:, :, :], wqk_d[layer, m, :, :, :])
                    psq = ps512.tile([128, T], F32, tag="ps512",
                                     name=f"psq_l{layer}_m{m}")
                    for i in range(DT):
                        nc.tensor.matmul(psq[:, :], wtile[:, i, :], h_bf[:, i, :],
                                         start=(i == 0), stop=(i == DT - 1))
                    # (Q + b) / sqrt(DH)
                    nc.vector.tensor_scalar(
                        qt[:, m, :], psq[:, :],
                        bias_sb[:, BC_QKVB + m:BC_QKVB + m + 1], 0.125,
                        ALU.add, ALU.mult)

                # ---- attention, head-pair software pipeline ----
                attnT = sb.tile([128, DT, T], BF16, tag="kta", bufs=2,
                                name=f"attnT_l{layer}")
                prev = None
                for hd in range(H + 1):
                    if hd < H:
                        r, hp = hd % 2, hd // 2
                        esm_list = []
                        for kt in range(8):
                            sps = ps512.tile([128, T], F32, tag="ps512",
                                             name=f"s_l{layer}_h{hd}_k{kt}")
                            nc.tensor.matmul(
                                sps[:, :],
                                ktall[64 * r:64 * r + 64, hp,
                                      128 * kt:128 * kt + 128],
                                qt[64 * r:64 * r + 64, hp, :],
                                start=True, stop=True)
                            es = sb.tile([128, T], BF16, tag="es", bufs=2,
                                         name=f"es_l{layer}_h{hd}_k{kt}")
                            nc.scalar.activation(es[:, :], sps[:, :], AF.Exp)
                            esm = sb.tile([128, T], BF16, tag=f"esm{hd % 2}",
                                          bufs=8,
                                          name=f"esm_l{layer}_h{hd}_k{kt}")
                            nc.vector.tensor_mul(esm[:, :], es[:, :],
                                                 mask_sb[:, kt, :])
                            esm_list.append(esm)
                    if prev is not None:
                        phd, plist = prev
                        pr, php = phd % 2, phd // 2
                        po = pso.tile([65, T], F32, tag="pso",
                                      name=f"po_l{layer}_h{phd}")
                        for kt in range(8):
                            nc.tensor.matmul(po[:, :], vall[:, kt, phd, :],
                                             plist[kt][:, :],
                                             start=(kt == 0), stop=(kt == 7))
                        rinv = rows.tile([1, T], F32, tag="rows",
                                         name=f"ainv_l{layer}_h{phd}")
                        nc.vector.reciprocal(rinv[:, :], po[64:65, :])
                        ibp = ps512.tile([64, T], F32, tag="ps512",
                                         name=f"aib_l{layer}_h{phd}")
                        nc.tensor.matmul(ibp[:, :], ones_f32[0:1, 0:64],
                                         rinv[:, :], start=True, stop=True)
                        ibs = sb.tile([64, T], F32, tag="ibs", bufs=1,
                                      name=f"aibs_l{layer}_h{phd}")
                        nc.scalar.copy(ibs[:, :], ibp[:, :])
                        nc.vector.tensor_mul(
                            attnT[64 * pr:64 * pr + 64, php, :],
                            po[0:64, :], ibs[:, :])
                    prev = (hd, esm_list) if hd < H else None

                # ---- attn output projection + residual ----
                for i in range(DT):
                    wtile = sb.tile([128, DT, 128], BF16, tag="wqk", bufs=3,
                                    name=f"wo_l{layer}_i{i}")
                    nc.sync.dma_start(wtile[:, :, :], wo_d[layer, i, :, :, :])
                    pao = ps512.tile([128, T], F32, tag="ps512",
                                     name=f"pao_l{layer}_i{i}")
                    for j in range(DT):
                        nc.tensor.matmul(pao[:, :], wtile[:, j, :],
                                         attnT[:, j, :],
                                         start=(j == 0), stop=(j == DT - 1))
                    # x = x + attn_out + b_O
                    nc.vector.scalar_tensor_tensor(
                        xT[:, i, :], pao[:, :],
                        bias_sb[:, BC_BO + i:BC_BO + i + 1], xT[:, i, :],
                        ALU.add, ALU.add)

                # ---- LN2 ----
                h2 = sb.tile([128, DT, T], BF16, tag="h", bufs=2,
                             name=f"h2_l{layer}")
                _layernorm(nc, sb, rows, ps512, lntmp, xT, bias_sb,
                           BC_L2W, BC_L2B, h2, ones_bf, ones_f32, eps_sb)

                # ---- MLP in + gelu ----
                gT = sb.tile([128, MT, T], BF16, tag="gT",
                             name=f"gT_l{layer}")
                for j in range(MT):
                    wtile = sb.tile([128, DT, 128], BF16, tag="wqk", bufs=3,
                                    name=f"wi_l{layer}_j{j}")
                    nc.sync.dma_start(wtile[:, :, :], win_d[layer, j, :, :, :])
                    pg = ps512.tile([128, T], F32, tag="ps512",
                                    name=f"pg_l{layer}_j{j}")
                    for i in range(DT):
                        nc.tensor.matmul(pg[:, :], wtile[:, i, :], h2[:, i, :],
                                         start=(i == 0), stop=(i == DT - 1))
                    nc.scalar.activation(gT[:, j, :], pg[:, :],
                                         AF.Gelu_apprx_tanh,
                                         bias=bias_sb[:, BC_BIN + j:BC_BIN + j + 1])

                # ---- MLP out + residual ----
                for i in range(DT):
                    wtile = sb.tile([128, MT, 128], BF16, tag="wout", bufs=2,
                                    name=f"wo2_l{layer}_i{i}")
                    nc.sync.dma_start(wtile[:, :, :], wout_d[layer, i, :, :, :])
                    pm = ps512.tile([128, T], F32, tag="ps512",
                                    name=f"pm_l{layer}_i{i}")
                    for j in range(MT):
                        nc.tensor.matmul(pm[:, :], wtile[:, j, :], gT[:, j, :],
                                         start=(j == 0), stop=(j == MT - 1))
                    nc.vector.scalar_tensor_tensor(
                        xT[:, i, :], pm[:, :],
                        bias_sb[:, BC_BOUT + i:BC_BOUT + i + 1], xT[:, i, :],
                        ALU.add, ALU.add)

            # ---- final LN ----
            lnf_sb = sb.tile([128, 2 * DT], F32, tag="lnf")
            nc.sync.dma_start(lnf_sb[:, :], lnf_d.ap())
            xf = sb.tile([128, DT, T], BF16, tag="h", bufs=2, name="xf")
            _layernorm(nc, sb, rows, ps512, lntmp, xT, lnf_sb, 0, DT, xf,
                       ones_bf, ones_f32, eps_sb)

            # ---- unembedding: logits[t, v] for all padded vocab slices ----
            for s in range(n_vslices):
                wu = sb.tile([128, DT, 512], BF16, tag="wu", bufs=3,
                             name=f"wu_s{s}")
                nc.sync.dma_start(wu[:, :, :], wu_d[s, :, :, :])
                for tt in range(4):
                    pu = ps512.tile([128, 512], F32, tag="ps512",
                                    name=f"pu_s{s}_t{tt}")
                    for i in range(DT):
                        nc.tensor.matmul(pu[:, :],
                                         xf[:, i, 128 * tt:128 * tt + 128],
                                         wu[:, i, :],
                                         start=(i == 0), stop=(i == DT - 1))
                    ou = sb.tile([128, 512], BF16, tag="ou", bufs=2,
                                 name=f"ou_s{s}_t{tt}")
                    if tt % 2 == 0:
                        nc.vector.tensor_copy(ou[:, :], pu[:, :])
                    else:
                        nc.scalar.copy(ou[:, :], pu[:, :])
                    nc.sync.dma_start(out_d[tt, :, s, :], ou[:, :])

    nc.compile()
    _BUILD_CACHE[key] = nc
    return nc


def _to_bf16(x):
    return np.ascontiguousarray(x.astype(ml_dtypes.bfloat16))


def prep_in_maps(inputs, n_layers=L, n_vslices=VS):
    """Host-side sharding: returns list of 8 per-core input dicts."""
    f = lambda k: np.asarray(inputs[k], dtype=np.float32)
    tokens = np.asarray(inputs["tokens"])
    W_E, W_pos = f("W_E"), f("W_pos")
    x_full = W_E[tokens] + W_pos[None, :S, :]        # [4, 1024, 768] f32

    nl = n_layers
    # fused QKV weight, feature-major lhsT layout [L, 128, DT, 2304]
    wq = f("W_Q").transpose(0, 2, 1, 3).reshape(L, D, D)[:nl]
    wk = f("W_K").transpose(0, 2, 1, 3).reshape(L, D, D)[:nl]
    wv = f("W_V").transpose(0, 2, 1, 3).reshape(L, D, D)[:nl]
    wqkc = np.concatenate([wq, wk], axis=2)           # [nl, 768, 1536]
    wqk = _to_bf16(wqkc.reshape(nl, DT, 128, 12, 128).transpose(0, 3, 2, 1, 4))
    wvp = _to_bf16(wv.reshape(nl, DT, 128, 2, 384).transpose(0, 3, 2, 1, 4))

    wo = f("W_O").reshape(L, D, D)[:nl]               # rows e = h*64+eh
    wo = _to_bf16(wo.reshape(nl, DT, 128, DT, 128).transpose(0, 3, 2, 1, 4))

    win = f("W_in")[:nl]                              # [nl, 768, 3072]
    win = _to_bf16(win.reshape(nl, DT, 128, MT, 128).transpose(0, 3, 2, 1, 4))

    wout = f("W_out")[:nl]                            # [nl, 3072, 768]
    wout = _to_bf16(wout.reshape(nl, MT, 128, DT, 128).transpose(0, 3, 2, 1, 4))

    wu_pad = np.zeros((D, VPAD), np.float32)
    wu_pad[:, :VOCAB] = f("W_U")
    wu = _to_bf16(wu_pad.reshape(DT, 128, VS, 512).transpose(2, 1, 0, 3))
    wu = np.ascontiguousarray(wu[:n_vslices])

    def percol(x, n):  # [nl, n*128] -> [nl, 128, n]
        return x.reshape(nl, n, 128).transpose(0, 2, 1)

    biases = np.zeros((nl, 128, BCOLS), np.float32)
    bq = f("b_Q").reshape(L, D)[:nl]
    bk = f("b_K").reshape(L, D)[:nl]
    biases[:, :, BC_QKVB:BC_QKVB + 12] = percol(
        np.concatenate([bq, bk], axis=1), 12)
    biases[:, :, BC_BO:BC_BO + DT] = percol(f("b_O")[:nl], DT)
    biases[:, :, BC_BIN:BC_BIN + MT] = percol(f("b_in")[:nl], MT)
    biases[:, :, BC_BOUT:BC_BOUT + DT] = percol(f("b_out")[:nl], DT)
    biases[:, :, BC_L1W:BC_L1W + DT] = percol(f("ln1_w")[:nl], DT)
    biases[:, :, BC_L1B:BC_L1B + DT] = percol(f("ln1_b")[:nl], DT)
    biases[:, :, BC_L2W:BC_L2W + DT] = percol(f("ln2_w")[:nl], DT)
    biases[:, :, BC_L2B:BC_L2B + DT] = percol(f("ln2_b")[:nl], DT)
    bv = f("b_V").reshape(L, D)[:nl]
    biases[:, :, BC_BV:BC_BV + D] = np.repeat(bv[:, None, :], 128, axis=1)

    lnf = np.zeros((128, 2 * DT), np.float32)
    lnf[:, 0:DT] = f("lnf_w").reshape(DT, 128).T
    lnf[:, DT:2 * DT] = f("lnf_b").reshape(DT, 128).T

    # per-parity causal mask: key(global)=128*kt+p  vs  query(global)=512*h+q
    kk = np.arange(128)[:, None, None]
    tt = np.arange(8)[None, :, None]
    qq = np.arange(T)[None, None, :]
    masks = []
    for h in range(2):
        m = (128 * tt + kk <= 512 * h + qq).astype(np.float32)
        masks.append(_to_bf16(m))

    in_maps = []
    for c in range(NCORES):
        b, h = c // 2, c % 2
        xh = x_full[b, T * h:T * (h + 1)]             # [512, 768]
        x0 = np.ascontiguousarray(
            xh.reshape(T, DT, 128).transpose(2, 1, 0)).astype(np.float32)
        in_maps.append({
            "x0": x0, "wqk": wqk, "wv": wvp, "wo": wo, "win": win,
            "wout": wout, "wu": wu, "biases": biases, "lnf": lnf,
            "mask": masks[h],
        })
    return in_maps


def assemble_output(results, inputs, n_vslices=VS):
    """results: list of 8 per-core out dicts -> full [4, 1024, VOCAB] f32."""
    vp = n_vslices * 512
    out = np.zeros((B, S, VOCAB), np.float32)
    for c in range(NCORES):
        b, h = c // 2, c % 2
        arr = np.asarray(results[c]["out"]).astype(np.float32)  # [4,128,vs,512]
        flat = arr.reshape(T, vp)[:, :min(vp, VOCAB)]
        out[b, T * h:T * h + T, :flat.shape[1]] = flat
    out += np.asarray(inputs["b_U"], dtype=np.float32)[None, None, :]
    return out


def install_trace_hook():
    """Register the axon NTFF profiling hook (missing from this image's
    antenv) so run_bass_kernel_spmd(trace=True) returns exec_time_ns."""
    import sys as _sys
    import types as _types
    import ctypes as _ctypes
    import contextlib as _contextlib
    if "antenv.axon_hooks" in _sys.modules:
        return

    def _make_hook():
        lib = _ctypes.CDLL("/opt/axon/libaxon_pjrt.so")
        if not hasattr(lib, "axon_start_nrt_profile"):
            return None
        lib.axon_start_nrt_profile.argtypes = [
            _ctypes.POINTER(_ctypes.c_int64), _ctypes.c_size_t]
        lib.axon_start_nrt_profile.restype = _ctypes.c_int64
        lib.axon_stop_nrt_profile.argtypes = [_ctypes.c_char_p]
        lib.axon_stop_nrt_profile.restype = _ctypes.c_int64

        @_contextlib.contextmanager
        def _hook(output_dir, device_ids):
            import jax
            jax.devices()
            if device_ids:
                ids = (_ctypes.c_int64 * len(device_ids))(*device_ids)
                rc = lib.axon_start_nrt_profile(ids, len(device_ids))
            else:
                rc = lib.axon_start_nrt_profile(None, 0)
            if rc != 0:
                raise RuntimeError(f"axon_start_nrt_profile rc={rc}")
            try:
                yield
            finally:
                lib.axon_stop_nrt_profile(str(output_dir).encode())
        return _hook

    mod = _types.ModuleType("antenv.axon_hooks")
    mod.get_axon_ntff_profile_hook = lambda: _make_hook()
    _sys.modules["antenv.axon_hooks"] = mod


def run(inputs, n_layers=L, n_vslices=VS, trace=False, tmpdir=None):
    """Build, run, and assemble. Returns (output, exec_time_ns)."""
    nc = build(n_layers, n_vslices)
    in_maps = prep_in_maps(inputs, n_layers, n_vslices)
    kwargs = {}
    if trace:
        install_trace_hook()
        tmpdir = tmpdir or "/tmp/bk_trace"
        import shutil
        shutil.rmtree(tmpdir, ignore_errors=True)
        os.makedirs(tmpdir, exist_ok=True)
        kwargs = dict(trace=True, tmpdir=tmpdir)
    res = bass_utils.run_bass_kernel_spmd(
        nc, in_maps, core_ids=list(range(NCORES)), **kwargs)
    out = assemble_output(res.results, inputs, n_vslices)
    return out, res.exec_time_ns


def kernel(**inputs):
    trace = bool(int(os.environ.get("BK_TRACE", "0")))
    out, t = run(inputs, trace=trace,
                 tmpdir=os.environ.get("BK_TRACE_DIR"))
    if trace:
        print(f"HW exec time: {t} ns")
    return out


# revision 12
# speedup vs baseline: 1.1607x; 1.0077x over previous
"""GPT-2 small forward pass on 8 TRN2 NeuronCores (Bass/Tile).

Sharding: 8 cores = 4 batch elements x 2 sequence halves (512 tokens each).
Each core runs the full 12-layer trunk on its 512 tokens with replicated
weights; the only cross-core traffic is a per-layer 2-core AllGather of K/V
within each batch pair. Attention uses a transposed-score layout (keys on
partitions, queries on free dim) so the softmax denominator falls out of a
ones-augmented V matmul; causal masking is a multiplicative {0,1} bf16 mask
passed as per-core input data. All matmuls bf16 with fp32 PSUM accumulation;
residual stream and layernorm statistics in fp32.

Host side: embedding gather (W_E[tokens] + W_pos), weight repacking/bf16
cast, final unshard + b_U add.

Self-contained: only numpy/ml_dtypes/concourse imports; all shapes hardcoded.
"""

import os
import numpy as np
import ml_dtypes

import concourse.bass as bass
import concourse.mybir as mybir
import concourse.tile as tile
from concourse import bacc
from concourse import bass_utils

F32 = mybir.dt.float32
BF16 = mybir.dt.bfloat16
AF = mybir.ActivationFunctionType
ALU = mybir.AluOpType

# model dims
B, S, D, H, DH, DM, L, VOCAB = 4, 1024, 768, 12, 64, 3072, 12, 50257
T = 512              # tokens per core
NCORES = 8
DT = D // 128        # 6  d-tiles
MT = DM // 128       # 24 m-tiles of d_mlp
VS = (VOCAB + 511) // 512   # 99 vocab slices
VPAD = VS * 512      # 50688
EPS = 1e-5
PAIRS = [[0, 1], [2, 3], [4, 5], [6, 7]]

# bias-pack columns inside the per-layer [128, 840] f32 tensor
BC_QKVB = 0     # 12 cols: Q then K feature-tile biases
BC_BO = 12      # 6
BC_BIN = 18     # 24
BC_BOUT = 42    # 6
BC_L1W = 48     # 6
BC_L1B = 54     # 6
BC_L2W = 60     # 6
BC_L2B = 66     # 6
BC_BV = 72      # 768 (host-replicated across partitions)
BCOLS = 840

_BUILD_CACHE = {}


class LNStats:
    """Incremental layernorm statistics: per-d-tile column sums of x and x^2
    accumulated into two PSUM rows. Emit stat_tile(i) right after xT[:, i, :]
    is finalized (inside the preceding residual-add loop) so the ones-matmuls
    interleave with that phase's main matmul stream instead of stalling PE."""

    def __init__(self, nc, sb, ps512, ones_bf, name):
        self.nc, self.sb, self.ones_bf = nc, sb, ones_bf
        self.s1 = ps512.tile([1, T], F32, tag="ps512", name=f"{name}_s1")
        self.s2 = ps512.tile([1, T], F32, tag="ps512", name=f"{name}_s2")
        self.name = name

    def stat_tile(self, x_sb, i):
        nc, sb = self.nc, self.sb
        xb = sb.tile([128, T], BF16, tag="lnxb", bufs=3,
                     name=f"{self.name}_xb{i}")
        nc.scalar.copy(xb[:, :], x_sb[:, i, :])
        xsq = sb.tile([128, T], BF16, tag="lnxsq", bufs=3,
                      name=f"{self.name}_xsq{i}")
        nc.vector.tensor_mul(xsq[:, :], xb[:, :], xb[:, :])
        nc.tensor.matmul(self.s1[:, :], self.ones_bf[:, 0:1], xb[:, :],
                         start=(i == 0), stop=(i == DT - 1))
        nc.tensor.matmul(self.s2[:, :], self.ones_bf[:, 0:1], xsq[:, :],
                         start=(i == 0), stop=(i == DT - 1))


def _ln_finish(nc, rows, ps512, lntmp, stats, x_sb, bias_sb, wcol, bcol,
               out_bf, ones_f32, eps_sb, n_feat=D):
    """Finish LN from accumulated stats: rows chain, broadcast, normalize.
    out_bf[:, i, :] lands per-slice so downstream matmuls on slice i can
    start before slice i+1 exists."""
    s1, s2, name = stats.s1, stats.s2, stats.name
    row_mean = rows.tile([1, T], F32, tag="rows", name=f"{name}_mean")
    nc.vector.tensor_scalar_mul(row_mean[:, :], s1[:, :], 1.0 / n_feat)
    mb = ps512.tile([128, T], F32, tag="ps512", name=f"{name}_mb")
    nc.tensor.matmul(mb[:, :], ones_f32[0:1, 0:128], row_mean[:, :],
                     start=True, stop=True)
    row_m2 = rows.tile([1, T], F32, tag="rows", name=f"{name}_m2")
    nc.vector.tensor_mul(row_m2[:, :], s1[:, :], row_mean[:, :])
    row_var = rows.tile([1, T], F32, tag="rows", name=f"{name}_var")
    nc.vector.tensor_sub(row_var[:, :], s2[:, :], row_m2[:, :])
    # unbiased variance (ddof=1): var = (sum_x2 - sum_x*mean) / (n-1)
    row_std = rows.tile([1, T], F32, tag="rows", name=f"{name}_std")
    nc.scalar.activation(row_std[:, :], row_var[:, :], AF.Sqrt,
                         bias=eps_sb[0:1, 0:1], scale=1.0 / (n_feat - 1))
    row_inv = rows.tile([1, T], F32, tag="rows", name=f"{name}_inv")
    nc.vector.reciprocal(row_inv[:, :], row_std[:, :])
    ib = ps512.tile([128, T], F32, tag="ps512", name=f"{name}_ib")
    nc.tensor.matmul(ib[:, :], ones_f32[0:1, 0:128], row_inv[:, :],
                     start=True, stop=True)

    for i in range(DT):
        t0 = lntmp.tile([128, T], F32, tag="lntmp", name=f"{name}_t{i}")
        nc.vector.tensor_sub(t0[:, :], x_sb[:, i, :], mb[:, :])
        u0 = lntmp.tile([128, T], F32, tag="lntmp", name=f"{name}_u{i}")
        nc.vector.tensor_mul(u0[:, :], t0[:, :], ib[:, :])
        nc.scalar.activation(out_bf[:, i, :], u0[:, :], AF.Identity,
                             bias=bias_sb[:, bcol + i:bcol + i + 1],
                             scale=bias_sb[:, wcol + i:wcol + i + 1])


def build(n_layers=L, n_vslices=VS):
    """Build + compile the SPMD kernel. Returns the Bacc object."""
    key = (n_layers, n_vslices)
    if key in _BUILD_CACHE:
        return _BUILD_CACHE[key]

    nc = bacc.Bacc("TRN2", target_bir_lowering=False, debug=False,
                   enable_asserts=False, num_devices=NCORES)

    # ---- kernel I/O (per-core shards; all cores same shapes) ----
    x0_d = nc.dram_tensor("x0", [128, DT, T], F32, kind="ExternalInput")
    wqk_d = nc.dram_tensor("wqk", [n_layers, 12, 128, DT, 128], BF16,
                           kind="ExternalInput")
    wv_d = nc.dram_tensor("wv", [n_layers, 2, 128, DT, 384], BF16,
                          kind="ExternalInput")
    wo_d = nc.dram_tensor("wo", [n_layers, DT, 128, DT, 128], BF16,
                          kind="ExternalInput")
    win_d = nc.dram_tensor("win", [n_layers, MT, 128, DT, 128], BF16,
                           kind="ExternalInput")
    wout_d = nc.dram_tensor("wout", [n_layers, DT, 128, MT, 128], BF16,
                            kind="ExternalInput")
    wu_d = nc.dram_tensor("wu", [n_vslices, 128, DT, 512], BF16,
                          kind="ExternalInput")
    bias_d = nc.dram_tensor("biases", [n_layers, 128, BCOLS], F32,
                            kind="ExternalInput")
    lnf_d = nc.dram_tensor("lnf", [128, 2 * DT], F32, kind="ExternalInput")
    mask_d = nc.dram_tensor("mask", [128, 8, T], BF16, kind="ExternalInput")
    out_d = nc.dram_tensor("out", [4, 128, n_vslices, 512], BF16,
                           kind="ExternalOutput")

    with tile.TileContext(nc) as tc:
        with tc.tile_pool(name="sb", bufs=1) as sb, \
             tc.tile_pool(name="rows", bufs=6) as rows, \
             tc.tile_pool(name="lntmp", bufs=2) as lntmp, \
             tc.tile_pool(name="ps512", bufs=4, space="PSUM") as ps512, \
             tc.tile_pool(name="pso", bufs=2, space="PSUM") as pso, \
             tc.tile_pool(name="dram", bufs=2, space="DRAM") as dram:

            # ---- persistent tiles ----
            ones_f32 = sb.tile([128, 128], F32, tag="ones_f32")
            nc.vector.memset(ones_f32[:, :], 1.0)
            ones_bf = sb.tile([128, 1], BF16, tag="ones_bf")
            nc.vector.memset(ones_bf[:, :], 1.0)
            eps_sb = sb.tile([128, 1], F32, tag="eps")
            nc.vector.memset(eps_sb[:, :], EPS)

            xT = sb.tile([128, DT, T], F32, tag="xT")
            nc.sync.dma_start(xT[:, :, :], x0_d.ap())
            stats_next = LNStats(nc, sb, ps512, ones_bf, "ln1_l0")
            for i in range(DT):
                stats_next.stat_tile(xT, i)

            mask_sb = sb.tile([128, 8, T], BF16, tag="mask")
            nc.sync.dma_start(mask_sb[:, :, :], mask_d.ap())

            # K/V of both sequence halves, in global token order
            ktall = sb.tile([128, DT, 2 * T], BF16, tag="ktall")
            vall = sb.tile([128, 8, H, 65], BF16, tag="vall")
            v_own = sb.tile([128, 4, H, 65], BF16, tag="vown")
            nc.vector.memset(v_own[:, :, :, 64:65], 1.0)  # denominator ones col

            for layer in range(n_layers):
                bias_sb = sb.tile([128, BCOLS], F32, tag="bias", bufs=2,
                                  name=f"bias_l{layer}")
                nc.sync.dma_start(bias_sb[:, :], bias_d[layer, :, :])

                # ---- LN1 (stats pre-accumulated in the previous phase) ----
                h_bf = sb.tile([128, DT, T], BF16, tag="h", bufs=2,
                               name=f"h1_l{layer}")
                _ln_finish(nc, rows, ps512, lntmp, stats_next, xT, bias_sb,
                           BC_L1W, BC_L1B, h_bf, ones_f32, eps_sb)

                # ---- K projection (feature-major KT) ----
                kt_own = sb.tile([128, DT, T], BF16, tag="kta", bufs=2,
                                 name=f"ktown_l{layer}")
                for m in range(6, 12):
                    wtile = sb.tile([128, DT, 128], BF16, tag="wqk", bufs=3,
                                    name=f"wk_l{layer}_m{m}")
                    nc.sync.dma_start(wtile[:, :, :], wqk_d[layer, m, :, :, :])
                    psq = ps512.tile([128, T], F32, tag="ps512",
                                     name=f"psk_l{layer}_m{m}")
                    for i in range(DT):
                        nc.tensor.matmul(psq[:, :], wtile[:, i, :], h_bf[:, i, :],
                                         start=(i == 0), stop=(i == DT - 1))
                    nc.scalar.activation(
                        kt_own[:, m - 6, :], psq[:, :], AF.Identity,
                        bias=bias_sb[:, BC_QKVB + m:BC_QKVB + m + 1])

                # ---- V projection (token-major, lhsT = h tiles) ----
                for half in range(2):
                    wv = sb.tile([128, DT, 384], BF16, tag="wv", bufs=2,
                                 name=f"wv_l{layer}_{half}")
                    nc.sync.dma_start(wv[:, :, :], wv_d[layer, half, :, :, :])
                    for tt in range(4):
                        psv = ps512.tile([128, 384], F32, tag="ps512",
                                         name=f"psv_l{layer}_{half}_{tt}")
                        for i in range(DT):
                            nc.tensor.matmul(psv[:, :],
                                             h_bf[:, i, 128 * tt:128 * tt + 128],
                                             wv[:, i, :],
                                             start=(i == 0), stop=(i == DT - 1))
                        nc.vector.tensor_add(
                            v_own[:, tt, 6 * half:6 * half + 6, 0:64], psv[:, :],
                            bias_sb[:, BC_BV + 384 * half:BC_BV + 384 * (half + 1)])

                # ---- pair exchange of K/V ----
                KTN = DT * T
                VN = 4 * H * 65
                bounce_in = dram.tile([128, KTN + VN], BF16, tag="cin",
                                      name=f"cin_l{layer}")
                bounce_out = dram.tile([256, KTN + VN], BF16, tag="cout",
                                       name=f"cout_l{layer}")
                nc.sync.dma_start(bounce_in[:, 0:KTN], kt_own[:, :, :])
                nc.sync.dma_start(bounce_in[:, KTN:KTN + VN],
                                  v_own[:, :, :, :])
                nc.gpsimd.collective_compute(
                    "AllGather", ALU.bypass, replica_groups=PAIRS,
                    ins=[bounce_in[:, :].opt()], outs=[bounce_out[:, :].opt()])
                for c in range(2):
                    nc.sync.dma_start(
                        ktall[:, :, T * c:T * (c + 1)],
                        bounce_out[128 * c:128 * c + 128, 0:KTN].rearrange(
                            "p (i t) -> p i t", i=DT))
                    nc.sync.dma_start(
                        vall[:, 4 * c:4 * (c + 1), :, :],
                        bounce_out[128 * c:128 * c + 128, KTN:KTN + VN])

                # ---- Q projection (overlaps the collective) ----
                qt = sb.tile([128, DT, T], BF16, tag="qt", bufs=1,
                             name=f"qt_l{layer}")
                for m in range(6):
                    wtile = sb.tile([128, DT, 128], BF16, tag="wqk", bufs=3,
                                    name=f"wq_l{layer}_m{m}")
                    nc.sync.dma_start(wtile[:, :, :], wqk_d[layer, m, :, :, :])
                    psq = ps512.tile([128, T], F32, tag="ps512",
                                     name=f"psq_l{layer}_m{m}")
                    for i in range(DT):
                        nc.tensor.matmul(psq[:, :], wtile[:, i, :], h_bf[:, i, :],
                                         start=(i == 0), stop=(i == DT - 1))
                    # (Q + b) / sqrt(DH); host stores b_Q / 8
                    nc.scalar.activation(
                        qt[:, m, :], psq[:, :], AF.Identity,
                        bias=bias_sb[:, BC_QKVB + m:BC_QKVB + m + 1],
                        scale=0.125)

                # ---- attention, head-pair software pipeline ----
                attnT = sb.tile([128, DT, T], BF16, tag="kta", bufs=2,
                                name=f"attnT_l{layer}")
                prev = None
                for hd in range(H + 1):
                    if hd < H:
                        r, hp = hd % 2, hd // 2
                        esm_list = []
                        for kt in range(8):
                            sps = ps512.tile([128, T], F32, tag="ps512",
                                             name=f"s_l{layer}_h{hd}_k{kt}")
                            nc.tensor.matmul(
                                sps[:, :],
                                ktall[64 * r:64 * r + 64, hp,
                                      128 * kt:128 * kt + 128],
                                qt[64 * r:64 * r + 64, hp, :],
                                start=True, stop=True)
                            es = sb.tile([128, T], BF16, tag="es", bufs=2,
                                         name=f"es_l{layer}_h{hd}_k{kt}")
                            nc.scalar.activation(es[:, :], sps[:, :], AF.Exp)
                            esm = sb.tile([128, T], BF16, tag=f"esm{hd % 2}",
                                          bufs=8,
                                          name=f"esm_l{layer}_h{hd}_k{kt}")
                            nc.vector.tensor_mul(esm[:, :], es[:, :],
                                                 mask_sb[:, kt, :])
                            esm_list.append(esm)
                    if prev is not None:
                        phd, plist = prev
                        pr, php = phd % 2, phd // 2
                        po = pso.tile([65, T], F32, tag="pso",
                                      name=f"po_l{layer}_h{phd}")
                        for kt in range(8):
                            nc.tensor.matmul(po[:, :], vall[:, kt, phd, :],
                                             plist[kt][:, :],
                                             start=(kt == 0), stop=(kt == 7))
                        rinv = rows.tile([1, T], F32, tag="rows",
                                         name=f"ainv_l{layer}_h{phd}")
                        nc.vector.reciprocal(rinv[:, :], po[64:65, :])
                        ibp = ps512.tile([64, T], F32, tag="ps512",
                                         name=f"aib_l{layer}_h{phd}")
                        nc.tensor.matmul(ibp[:, :], ones_f32[0:1, 0:64],
                                         rinv[:, :], start=True, stop=True)
                        ibs = sb.tile([64, T], F32, tag="ibs", bufs=1,
                                      name=f"aibs_l{layer}_h{phd}")
                        nc.scalar.copy(ibs[:, :], ibp[:, :])
                        nc.vector.tensor_mul(
                            attnT[64 * pr:64 * pr + 64, php, :],
                            po[0:64, :], ibs[:, :])
                    prev = (hd, esm_list) if hd < H else None

                # ---- attn output projection + residual ----
                stats2 = LNStats(nc, sb, ps512, ones_bf, f"ln2_l{layer}")
                for i in range(DT):
                    wtile = sb.tile([128, DT, 128], BF16, tag="wqk", bufs=3,
                                    name=f"wo_l{layer}_i{i}")
                    nc.sync.dma_start(wtile[:, :, :], wo_d[layer, i, :, :, :])
                    pao = ps512.tile([128, T], F32, tag="ps512",
                                     name=f"pao_l{layer}_i{i}")
                    for j in range(DT):
                        nc.tensor.matmul(pao[:, :], wtile[:, j, :],
                                         attnT[:, j, :],
                                         start=(j == 0), stop=(j == DT - 1))
                    # x = x + attn_out + b_O
                    nc.vector.scalar_tensor_tensor(
                        xT[:, i, :], pao[:, :],
                        bias_sb[:, BC_BO + i:BC_BO + i + 1], xT[:, i, :],
                        ALU.add, ALU.add)
                    stats2.stat_tile(xT, i)

                # ---- LN2 ----
                h2 = sb.tile([128, DT, T], BF16, tag="h", bufs=2,
                             name=f"h2_l{layer}")
                _ln_finish(nc, rows, ps512, lntmp, stats2, xT, bias_sb,
                           BC_L2W, BC_L2B, h2, ones_f32, eps_sb)

                # ---- MLP in + gelu ----
                gT = sb.tile([128, MT, T], BF16, tag="gT",
                             name=f"gT_l{layer}")
                for j in range(MT):
                    wtile = sb.tile([128, DT, 128], BF16, tag="wqk", bufs=3,
                                    name=f"wi_l{layer}_j{j}")
                    nc.sync.dma_start(wtile[:, :, :], win_d[layer, j, :, :, :])
                    pg = ps512.tile([128, T], F32, tag="ps512",
                                    name=f"pg_l{layer}_j{j}")
                    for i in range(DT):
                        nc.tensor.matmul(pg[:, :], wtile[:, i, :], h2[:, i, :],
                                         start=(i == 0), stop=(i == DT - 1))
                    nc.scalar.activation(gT[:, j, :], pg[:, :],
                                         AF.Gelu_apprx_tanh,
                                         bias=bias_sb[:, BC_BIN + j:BC_BIN + j + 1])

                # ---- MLP out + residual ----
                stats_next = LNStats(nc, sb, ps512, ones_bf,
                                     f"ln1_l{layer + 1}")
                for i in range(DT):
                    wtile = sb.tile([128, MT, 128], BF16, tag="wout", bufs=2,
                                    name=f"wo2_l{layer}_i{i}")
                    nc.sync.dma_start(wtile[:, :, :], wout_d[layer, i, :, :, :])
                    pm = ps512.tile([128, T], F32, tag="ps512",
                                    name=f"pm_l{layer}_i{i}")
                    for j in range(MT):
                        nc.tensor.matmul(pm[:, :], wtile[:, j, :], gT[:, j, :],
                                         start=(j == 0), stop=(j == MT - 1))
                    nc.vector.scalar_tensor_tensor(
                        xT[:, i, :], pm[:, :],
                        bias_sb[:, BC_BOUT + i:BC_BOUT + i + 1], xT[:, i, :],
                        ALU.add, ALU.add)
                    stats_next.stat_tile(xT, i)

            # ---- final LN ----
            lnf_sb = sb.tile([128, 2 * DT], F32, tag="lnf")
            nc.sync.dma_start(lnf_sb[:, :], lnf_d.ap())
            xf = sb.tile([128, DT, T], BF16, tag="h", bufs=2, name="xf")
            _ln_finish(nc, rows, ps512, lntmp, stats_next, xT, lnf_sb, 0, DT,
                       xf, ones_f32, eps_sb)

            # ---- unembedding: logits[t, v] for all padded vocab slices ----
            for s in range(n_vslices):
                wu = sb.tile([128, DT, 512], BF16, tag="wu", bufs=3,
                             name=f"wu_s{s}")
                nc.sync.dma_start(wu[:, :, :], wu_d[s, :, :, :])
                for tt in range(4):
                    pu = ps512.tile([128, 512], F32, tag="ps512",
                                    name=f"pu_s{s}_t{tt}")
                    for i in range(DT):
                        nc.tensor.matmul(pu[:, :],
                                         xf[:, i, 128 * tt:128 * tt + 128],
                                         wu[:, i, :],
                                         start=(i == 0), stop=(i == DT - 1))
                    ou = sb.tile([128, 512], BF16, tag="ou", bufs=2,
                                 name=f"ou_s{s}_t{tt}")
                    if tt % 2 == 0:
                        nc.vector.tensor_copy(ou[:, :], pu[:, :])
                    else:
                        nc.scalar.copy(ou[:, :], pu[:, :])
                    nc.sync.dma_start(out_d[tt, :, s, :], ou[:, :])

    nc.compile()
    _BUILD_CACHE[key] = nc
    return nc


def _to_bf16(x):
    return np.ascontiguousarray(x.astype(ml_dtypes.bfloat16))


def prep_in_maps(inputs, n_layers=L, n_vslices=VS):
    """Host-side sharding: returns list of 8 per-core input dicts."""
    f = lambda k: np.asarray(inputs[k], dtype=np.float32)
    tokens = np.asarray(inputs["tokens"])
    W_E, W_pos = f("W_E"), f("W_pos")
    x_full = W_E[tokens] + W_pos[None, :S, :]        # [4, 1024, 768] f32

    nl = n_layers
    # fused QKV weight, feature-major lhsT layout [L, 128, DT, 2304]
    wq = f("W_Q").transpose(0, 2, 1, 3).reshape(L, D, D)[:nl]
    wk = f("W_K").transpose(0, 2, 1, 3).reshape(L, D, D)[:nl]
    wv = f("W_V").transpose(0, 2, 1, 3).reshape(L, D, D)[:nl]
    wqkc = np.concatenate([wq, wk], axis=2)           # [nl, 768, 1536]
    wqk = _to_bf16(wqkc.reshape(nl, DT, 128, 12, 128).transpose(0, 3, 2, 1, 4))
    wvp = _to_bf16(wv.reshape(nl, DT, 128, 2, 384).transpose(0, 3, 2, 1, 4))

    wo = f("W_O").reshape(L, D, D)[:nl]               # rows e = h*64+eh
    wo = _to_bf16(wo.reshape(nl, DT, 128, DT, 128).transpose(0, 3, 2, 1, 4))

    win = f("W_in")[:nl]                              # [nl, 768, 3072]
    win = _to_bf16(win.reshape(nl, DT, 128, MT, 128).transpose(0, 3, 2, 1, 4))

    wout = f("W_out")[:nl]                            # [nl, 3072, 768]
    wout = _to_bf16(wout.reshape(nl, MT, 128, DT, 128).transpose(0, 3, 2, 1, 4))

    wu_pad = np.zeros((D, VPAD), np.float32)
    wu_pad[:, :VOCAB] = f("W_U")
    wu = _to_bf16(wu_pad.reshape(DT, 128, VS, 512).transpose(2, 1, 0, 3))
    wu = np.ascontiguousarray(wu[:n_vslices])

    def percol(x, n):  # [nl, n*128] -> [nl, 128, n]
        return x.reshape(nl, n, 128).transpose(0, 2, 1)

    biases = np.zeros((nl, 128, BCOLS), np.float32)
    bq = f("b_Q").reshape(L, D)[:nl]
    bk = f("b_K").reshape(L, D)[:nl]
    biases[:, :, BC_QKVB:BC_QKVB + 12] = percol(
        np.concatenate([bq * 0.125, bk], axis=1), 12)
    biases[:, :, BC_BO:BC_BO + DT] = percol(f("b_O")[:nl], DT)
    biases[:, :, BC_BIN:BC_BIN + MT] = percol(f("b_in")[:nl], MT)
    biases[:, :, BC_BOUT:BC_BOUT + DT] = percol(f("b_out")[:nl], DT)
    biases[:, :, BC_L1W:BC_L1W + DT] = percol(f("ln1_w")[:nl], DT)
    biases[:, :, BC_L1B:BC_L1B + DT] = percol(f("ln1_b")[:nl], DT)
    biases[:, :, BC_L2W:BC_L2W + DT] = percol(f("ln2_w")[:nl], DT)
    biases[:, :, BC_L2B:BC_L2B + DT] = percol(f("ln2_b")[:nl], DT)
    bv = f("b_V").reshape(L, D)[:nl]
    biases[:, :, BC_BV:BC_BV + D] = np.repeat(bv[:, None, :], 128, axis=1)

    lnf = np.zeros((128, 2 * DT), np.float32)
    lnf[:, 0:DT] = f("lnf_w").reshape(DT, 128).T
    lnf[:, DT:2 * DT] = f("lnf_b").reshape(DT, 128).T

    # per-parity causal mask: key(global)=128*kt+p  vs  query(global)=512*h+q
    kk = np.arange(128)[:, None, None]
    tt = np.arange(8)[None, :, None]
    qq = np.arange(T)[None, None, :]
    masks = []
    for h in range(2):
        m = (128 * tt + kk <= 512 * h + qq).astype(np.float32)
        masks.append(_to_bf16(m))

    in_maps = []
    for c in range(NCORES):
        b, h = c // 2, c % 2
        xh = x_full[b, T * h:T * (h + 1)]             # [512, 768]
        x0 = np.ascontiguousarray(
            xh.reshape(T, DT, 128).transpose(2, 1, 0)).astype(np.float32)
        in_maps.append({
            "x0": x0, "wqk": wqk, "wv": wvp, "wo": wo, "win": win,
            "wout": wout, "wu": wu, "biases": biases, "lnf": lnf,
            "mask": masks[h],
        })
    return in_maps


def assemble_output(results, inputs, n_vslices=VS):
    """results: list of 8 per-core out dicts -> full [4, 1024, VOCAB] f32."""
    vp = n_vslices * 512
    out = np.zeros((B, S, VOCAB), np.float32)
    for c in range(NCORES):
        b, h = c // 2, c % 2
        arr = np.asarray(results[c]["out"]).astype(np.float32)  # [4,128,vs,512]
        flat = arr.reshape(T, vp)[:, :min(vp, VOCAB)]
        out[b, T * h:T * h + T, :flat.shape[1]] = flat
    out += np.asarray(inputs["b_U"], dtype=np.float32)[None, None, :]
    return out


def install_trace_hook():
    """Register the axon NTFF profiling hook (missing from this image's
    antenv) so run_bass_kernel_spmd(trace=True) returns exec_time_ns."""
    import sys as _sys
    import types as _types
    import ctypes as _ctypes
    import contextlib as _contextlib
    if "antenv.axon_hooks" in _sys.modules:
        return

    def _make_hook():
        lib = _ctypes.CDLL("/opt/axon/libaxon_pjrt.so")
        if not hasattr(lib, "axon_start_nrt_profile"):
            return None
        lib.axon_start_nrt_profile.argtypes = [
            _ctypes.POINTER(_ctypes.c_int64), _ctypes.c_size_t]
        lib.axon_start_nrt_profile.restype = _ctypes.c_int64
        lib.axon_stop_nrt_profile.argtypes = [_ctypes.c_char_p]
        lib.axon_stop_nrt_profile.restype = _ctypes.c_int64

        @_contextlib.contextmanager
        def _hook(output_dir, device_ids):
            import jax
            jax.devices()
            if device_ids:
                ids = (_ctypes.c_int64 * len(device_ids))(*device_ids)
                rc = lib.axon_start_nrt_profile(ids, len(device_ids))
            else:
                rc = lib.axon_start_nrt_profile(None, 0)
            if rc != 0:
                raise RuntimeError(f"axon_start_nrt_profile rc={rc}")
            try:
                yield
            finally:
                lib.axon_stop_nrt_profile(str(output_dir).encode())
        return _hook

    mod = _types.ModuleType("antenv.axon_hooks")
    mod.get_axon_ntff_profile_hook = lambda: _make_hook()
    _sys.modules["antenv.axon_hooks"] = mod


def run(inputs, n_layers=L, n_vslices=VS, trace=False, tmpdir=None):
    """Build, run, and assemble. Returns (output, exec_time_ns)."""
    nc = build(n_layers, n_vslices)
    in_maps = prep_in_maps(inputs, n_layers, n_vslices)
    kwargs = {}
    if trace:
        install_trace_hook()
        tmpdir = tmpdir or "/tmp/bk_trace"
        import shutil
        shutil.rmtree(tmpdir, ignore_errors=True)
        os.makedirs(tmpdir, exist_ok=True)
        kwargs = dict(trace=True, tmpdir=tmpdir)
    res = bass_utils.run_bass_kernel_spmd(
        nc, in_maps, core_ids=list(range(NCORES)), **kwargs)
    out = assemble_output(res.results, inputs, n_vslices)
    return out, res.exec_time_ns


def kernel(**inputs):
    trace = bool(int(os.environ.get("BK_TRACE", "0")))
    out, t = run(inputs, trace=trace,
                 tmpdir=os.environ.get("BK_TRACE_DIR"))
    if trace:
        print(f"HW exec time: {t} ns")
    return out


# revision 15
# speedup vs baseline: 1.2355x; 1.0645x over previous
"""GPT-2 small forward pass on 8 TRN2 NeuronCores (Bass/Tile).

Sharding: 8 cores = 4 batch elements x 2 sequence halves (512 tokens each).
Each core runs the full 12-layer trunk on its 512 tokens with replicated
weights; the only cross-core traffic is a per-layer 2-core AllGather of K/V
within each batch pair. Attention uses a transposed-score layout (keys on
partitions, queries on free dim) so the softmax denominator falls out of a
ones-augmented V matmul; causal masking is a multiplicative {0,1} bf16 mask
passed as per-core input data. All matmuls bf16 with fp32 PSUM accumulation;
residual stream and layernorm statistics in fp32.

Host side: embedding gather (W_E[tokens] + W_pos), weight repacking/bf16
cast, final unshard + b_U add.

Self-contained: only numpy/ml_dtypes/concourse imports; all shapes hardcoded.
"""

import os
import numpy as np
import ml_dtypes

import concourse.bass as bass
import concourse.mybir as mybir
import concourse.tile as tile
from concourse import bacc
from concourse import bass_utils

F32 = mybir.dt.float32
BF16 = mybir.dt.bfloat16
AF = mybir.ActivationFunctionType
ALU = mybir.AluOpType

# model dims
B, S, D, H, DH, DM, L, VOCAB = 4, 1024, 768, 12, 64, 3072, 12, 50257
T = 512              # tokens per core
NCORES = 8
DT = D // 128        # 6  d-tiles
MT = DM // 128       # 24 m-tiles of d_mlp
VS = (VOCAB + 511) // 512   # 99 vocab slices
VPAD = VS * 512      # 50688
EPS = 1e-5
PAIRS = [[0, 1], [2, 3], [4, 5], [6, 7]]

# bias-pack columns inside the per-layer [128, 840] f32 tensor
BC_QKVB = 0     # 12 cols: Q then K feature-tile biases
BC_BO = 12      # 6
BC_BIN = 18     # 24
BC_BOUT = 42    # 6
BC_L1W = 48     # 6
BC_L1B = 54     # 6
BC_L2W = 60     # 6
BC_L2B = 66     # 6
BC_BV = 72      # 768 (host-replicated across partitions)
BCOLS = 840

_BUILD_CACHE = {}


class LNStats:
    """Incremental layernorm statistics: per-d-tile column sums of x and x^2
    accumulated into two PSUM rows. Emit stat_tile(i) right after xT[:, i, :]
    is finalized (inside the preceding residual-add loop) so the ones-matmuls
    interleave with that phase's main matmul stream instead of stalling PE."""

    def __init__(self, nc, sb, ps512, ones_bf, name):
        self.nc, self.sb, self.ones_bf = nc, sb, ones_bf
        self.s1 = ps512.tile([1, T], F32, tag="ps512", name=f"{name}_s1")
        self.s2 = ps512.tile([1, T], F32, tag="ps512", name=f"{name}_s2")
        self.name = name

    def stat_tile(self, x_sb, i):
        nc, sb = self.nc, self.sb
        xb = sb.tile([128, T], BF16, tag="lnxb", bufs=3,
                     name=f"{self.name}_xb{i}")
        nc.scalar.copy(xb[:, :], x_sb[:, i, :])
        xsq = sb.tile([128, T], BF16, tag="lnxsq", bufs=3,
                      name=f"{self.name}_xsq{i}")
        nc.vector.tensor_mul(xsq[:, :], xb[:, :], xb[:, :])
        nc.tensor.matmul(self.s1[:, :], self.ones_bf[:, 0:1], xb[:, :],
                         start=(i == 0), stop=(i == DT - 1))
        nc.tensor.matmul(self.s2[:, :], self.ones_bf[:, 0:1], xsq[:, :],
                         start=(i == 0), stop=(i == DT - 1))


def _ln_finish(nc, rows, ps512, lntmp, stats, x_sb, bias_sb, wcol, bcol,
               out_bf, ones_f32, eps_sb, n_feat=D):
    """Finish LN from accumulated stats: rows chain, broadcast, normalize.
    out_bf[:, i, :] lands per-slice so downstream matmuls on slice i can
    start before slice i+1 exists."""
    s1, s2, name = stats.s1, stats.s2, stats.name
    row_mean = rows.tile([1, T], F32, tag="rows", name=f"{name}_mean")
    nc.vector.tensor_scalar_mul(row_mean[:, :], s1[:, :], 1.0 / n_feat)
    row_mean_bf = rows.tile([1, T], BF16, tag="rows", name=f"{name}_meanb")
    nc.scalar.copy(row_mean_bf[:, :], row_mean[:, :])
    mb = ps512.tile([128, T], F32, tag="ps512", name=f"{name}_mb")
    nc.tensor.matmul(mb[:, :], stats.ones_bf[0:1, 0:128], row_mean_bf[:, :],
                     start=True, stop=True)
    row_m2 = rows.tile([1, T], F32, tag="rows", name=f"{name}_m2")
    nc.vector.tensor_mul(row_m2[:, :], s1[:, :], row_mean[:, :])
    row_var = rows.tile([1, T], F32, tag="rows", name=f"{name}_var")
    nc.vector.tensor_sub(row_var[:, :], s2[:, :], row_m2[:, :])
    # unbiased variance (ddof=1): var = (sum_x2 - sum_x*mean) / (n-1)
    row_std = rows.tile([1, T], F32, tag="rows", name=f"{name}_std")
    nc.scalar.activation(row_std[:, :], row_var[:, :], AF.Sqrt,
                         bias=eps_sb[0:1, 0:1], scale=1.0 / (n_feat - 1))
    row_inv = rows.tile([1, T], F32, tag="rows", name=f"{name}_inv")
    nc.vector.reciprocal(row_inv[:, :], row_std[:, :])
    row_inv_bf = rows.tile([1, T], BF16, tag="rows", name=f"{name}_invb")
    nc.scalar.copy(row_inv_bf[:, :], row_inv[:, :])
    ib = ps512.tile([128, T], F32, tag="ps512", name=f"{name}_ib")
    nc.tensor.matmul(ib[:, :], stats.ones_bf[0:1, 0:128], row_inv_bf[:, :],
                     start=True, stop=True)

    for i in range(DT):
        t0 = lntmp.tile([128, T], F32, tag="lntmp", name=f"{name}_t{i}")
        nc.vector.tensor_sub(t0[:, :], x_sb[:, i, :], mb[:, :])
        u0 = lntmp.tile([128, T], F32, tag="lntmp", name=f"{name}_u{i}")
        nc.vector.tensor_mul(u0[:, :], t0[:, :], ib[:, :])
        nc.scalar.activation(out_bf[:, i, :], u0[:, :], AF.Identity,
                             bias=bias_sb[:, bcol + i:bcol + i + 1],
                             scale=bias_sb[:, wcol + i:wcol + i + 1])


def build(n_layers=L, n_vslices=VS):
    """Build + compile the SPMD kernel. Returns the Bacc object."""
    key = (n_layers, n_vslices)
    if key in _BUILD_CACHE:
        return _BUILD_CACHE[key]

    nc = bacc.Bacc("TRN2", target_bir_lowering=False, debug=False,
                   enable_asserts=False, num_devices=NCORES)

    # ---- kernel I/O (per-core shards; all cores same shapes) ----
    x0_d = nc.dram_tensor("x0", [128, DT, T], F32, kind="ExternalInput")
    wqk_d = nc.dram_tensor("wqk", [n_layers, 12, 128, DT, 128], BF16,
                           kind="ExternalInput")
    wv_d = nc.dram_tensor("wv", [n_layers, 2, 128, DT, 384], BF16,
                          kind="ExternalInput")
    wo_d = nc.dram_tensor("wo", [n_layers, DT, 128, DT, 128], BF16,
                          kind="ExternalInput")
    win_d = nc.dram_tensor("win", [n_layers, MT, 128, DT, 128], BF16,
                           kind="ExternalInput")
    wout_d = nc.dram_tensor("wout", [n_layers, DT, 128, MT, 128], BF16,
                            kind="ExternalInput")
    wu_d = nc.dram_tensor("wu", [n_vslices, 128, DT, 512], BF16,
                          kind="ExternalInput")
    bias_d = nc.dram_tensor("biases", [n_layers, 128, BCOLS], F32,
                            kind="ExternalInput")
    lnf_d = nc.dram_tensor("lnf", [128, 2 * DT], F32, kind="ExternalInput")
    mask_d = nc.dram_tensor("mask", [128, 8, T], BF16, kind="ExternalInput")
    out_d = nc.dram_tensor("out", [4, 128, n_vslices, 512], BF16,
                           kind="ExternalOutput")

    with tile.TileContext(nc) as tc:
        with tc.tile_pool(name="sb", bufs=1) as sb, \
             tc.tile_pool(name="rows", bufs=6) as rows, \
             tc.tile_pool(name="lntmp", bufs=2) as lntmp, \
             tc.tile_pool(name="ps512", bufs=5, space="PSUM") as ps512, \
             tc.tile_pool(name="pso", bufs=3, space="PSUM") as pso, \
             tc.tile_pool(name="dram", bufs=2, space="DRAM") as dram:

            # ---- persistent tiles ----
            ones_f32 = sb.tile([128, 128], F32, tag="ones_f32")
            nc.vector.memset(ones_f32[:, :], 1.0)
            ones_bf = sb.tile([128, 128], BF16, tag="ones_bf")
            nc.vector.memset(ones_bf[:, :], 1.0)
            eps_sb = sb.tile([128, 1], F32, tag="eps")
            nc.vector.memset(eps_sb[:, :], EPS)

            xT = sb.tile([128, DT, T], F32, tag="xT")
            nc.sync.dma_start(xT[:, :, :], x0_d.ap())
            stats_next = LNStats(nc, sb, ps512, ones_bf, "ln1_l0")
            for i in range(DT):
                stats_next.stat_tile(xT, i)

            mask_sb = sb.tile([128, 8, T], BF16, tag="mask")
            nc.sync.dma_start(mask_sb[:, :, :], mask_d.ap())

            # K/V of both sequence halves, in global token order
            ktall = sb.tile([128, DT, 2 * T], BF16, tag="ktall")
            vall = sb.tile([128, 8, H, 65], BF16, tag="vall")
            v_own = sb.tile([128, 4, H, 65], BF16, tag="vown")
            nc.vector.memset(v_own[:, :, :, 64:65], 1.0)  # denominator ones col

            for layer in range(n_layers):
                bias_sb = sb.tile([128, BCOLS], F32, tag="bias", bufs=2,
                                  name=f"bias_l{layer}")
                nc.sync.dma_start(bias_sb[:, :], bias_d[layer, :, :])

                # ---- LN1 (stats pre-accumulated in the previous phase) ----
                h_bf = sb.tile([128, DT, T], BF16, tag="h", bufs=2,
                               name=f"h1_l{layer}")
                _ln_finish(nc, rows, ps512, lntmp, stats_next, xT, bias_sb,
                           BC_L1W, BC_L1B, h_bf, ones_f32, eps_sb)

                # ---- K projection (feature-major KT) ----
                kt_own = sb.tile([128, DT, T], BF16, tag="kta", bufs=2,
                                 name=f"ktown_l{layer}")
                for m in range(6, 12):
                    wtile = sb.tile([128, DT, 128], BF16, tag="wqk", bufs=3,
                                    name=f"wk_l{layer}_m{m}")
                    nc.sync.dma_start(wtile[:, :, :], wqk_d[layer, m, :, :, :])
                    psq = ps512.tile([128, T], F32, tag="ps512",
                                     name=f"psk_l{layer}_m{m}")
                    for i in range(DT):
                        nc.tensor.matmul(psq[:, :], wtile[:, i, :], h_bf[:, i, :],
                                         start=(i == 0), stop=(i == DT - 1))
                    nc.scalar.activation(
                        kt_own[:, m - 6, :], psq[:, :], AF.Identity,
                        bias=bias_sb[:, BC_QKVB + m:BC_QKVB + m + 1])

                # ---- V projection (token-major, lhsT = h tiles) ----
                for half in range(2):
                    wv = sb.tile([128, DT, 384], BF16, tag="wv", bufs=2,
                                 name=f"wv_l{layer}_{half}")
                    nc.sync.dma_start(wv[:, :, :], wv_d[layer, half, :, :, :])
                    for tt in range(4):
                        psv = ps512.tile([128, 384], F32, tag="ps512",
                                         name=f"psv_l{layer}_{half}_{tt}")
                        for i in range(DT):
                            nc.tensor.matmul(psv[:, :],
                                             h_bf[:, i, 128 * tt:128 * tt + 128],
                                             wv[:, i, :],
                                             start=(i == 0), stop=(i == DT - 1))
                        nc.vector.tensor_add(
                            v_own[:, tt, 6 * half:6 * half + 6, 0:64], psv[:, :],
                            bias_sb[:, BC_BV + 384 * half:BC_BV + 384 * (half + 1)])

                # ---- pair exchange of K/V ----
                KTN = DT * T
                VN = 4 * H * 65
                bounce_in = dram.tile([128, KTN + VN], BF16, tag="cin",
                                      name=f"cin_l{layer}")
                bounce_out = dram.tile([256, KTN + VN], BF16, tag="cout",
                                       name=f"cout_l{layer}")
                nc.sync.dma_start(bounce_in[:, 0:KTN], kt_own[:, :, :])
                nc.sync.dma_start(bounce_in[:, KTN:KTN + VN],
                                  v_own[:, :, :, :])
                nc.gpsimd.collective_compute(
                    "AllGather", ALU.bypass, replica_groups=PAIRS,
                    ins=[bounce_in[:, :].opt()], outs=[bounce_out[:, :].opt()])
                for c in range(2):
                    nc.sync.dma_start(
                        ktall[:, :, T * c:T * (c + 1)],
                        bounce_out[128 * c:128 * c + 128, 0:KTN].rearrange(
                            "p (i t) -> p i t", i=DT))
                    nc.sync.dma_start(
                        vall[:, 4 * c:4 * (c + 1), :, :],
                        bounce_out[128 * c:128 * c + 128, KTN:KTN + VN])

                # ---- Q projection (overlaps the collective) ----
                qt = sb.tile([128, DT, T], BF16, tag="qt", bufs=1,
                             name=f"qt_l{layer}")
                for m in range(6):
                    wtile = sb.tile([128, DT, 128], BF16, tag="wqk", bufs=3,
                                    name=f"wq_l{layer}_m{m}")
                    nc.sync.dma_start(wtile[:, :, :], wqk_d[layer, m, :, :, :])
                    psq = ps512.tile([128, T], F32, tag="ps512",
                                     name=f"psq_l{layer}_m{m}")
                    for i in range(DT):
                        nc.tensor.matmul(psq[:, :], wtile[:, i, :], h_bf[:, i, :],
                                         start=(i == 0), stop=(i == DT - 1))
                    # (Q + b) / sqrt(DH); host stores b_Q / 8
                    nc.scalar.activation(
                        qt[:, m, :], psq[:, :], AF.Identity,
                        bias=bias_sb[:, BC_QKVB + m:BC_QKVB + m + 1],
                        scale=0.125)

                # ---- attention, head-pair software pipeline ----
                attnT = sb.tile([128, DT, T], BF16, tag="kta", bufs=2,
                                name=f"attnT_l{layer}")
                # Heads 2hp (rows 0:64) and 2hp+1 (rows 64:128) issue adjacent
                # score matmuls that run concurrently in distinct row groups.
                # PV matmuls for k-tile kt-1 interleave with scores of kt.
                for hp in range(H // 2):
                    ha, hb = 2 * hp, 2 * hp + 1
                    po_a = pso.tile([65, T], F32, tag="pso",
                                    name=f"po_l{layer}_h{ha}")
                    po_b = pso.tile([65, T], F32, tag="pso",
                                    name=f"po_l{layer}_h{hb}")
                    esm_prev = None
                    for kt in range(8):
                        ks = slice(128 * kt, 128 * kt + 128)
                        sps_a = ps512.tile([128, T], F32, tag="ps512",
                                           name=f"s_l{layer}_h{ha}_k{kt}")
                        nc.tensor.matmul(sps_a[:, :], ktall[0:64, hp, ks],
                                         qt[0:64, hp, :],
                                         start=True, stop=True)
                        sps_b = ps512.tile([128, T], F32, tag="ps512",
                                           name=f"s_l{layer}_h{hb}_k{kt}")
                        nc.tensor.matmul(sps_b[:, :], ktall[64:128, hp, ks],
                                         qt[64:128, hp, :],
                                         start=True, stop=True)
                        if esm_prev is not None:
                            pk = kt - 1
                            nc.tensor.matmul(po_a[:, :], vall[:, pk, ha, :],
                                             esm_prev[0][:, :],
                                             start=(pk == 0), stop=False)
                            nc.tensor.matmul(po_b[:, :], vall[:, pk, hb, :],
                                             esm_prev[1][:, :],
                                             start=(pk == 0), stop=False)
                        pair = []
                        for hd, sps in ((ha, sps_a), (hb, sps_b)):
                            es = sb.tile([128, T], BF16, tag="es", bufs=3,
                                         name=f"es_l{layer}_h{hd}_k{kt}")
                            nc.scalar.activation(es[:, :], sps[:, :], AF.Exp)
                            esm = sb.tile([128, T], BF16, tag="esm", bufs=6,
                                          name=f"esm_l{layer}_h{hd}_k{kt}")
                            nc.vector.tensor_mul(esm[:, :], es[:, :],
                                                 mask_sb[:, kt, :])
                            pair.append(esm)
                        esm_prev = pair
                    nc.tensor.matmul(po_a[:, :], vall[:, 7, ha, :],
                                     esm_prev[0][:, :], start=False, stop=True)
                    nc.tensor.matmul(po_b[:, :], vall[:, 7, hb, :],
                                     esm_prev[1][:, :], start=False, stop=True)
                    for r, po in ((0, po_a), (1, po_b)):
                        rinv = rows.tile([1, T], F32, tag="rows",
                                         name=f"ainv_l{layer}_p{hp}_{r}")
                        nc.vector.reciprocal(rinv[:, :], po[64:65, :])
                        rinvb = rows.tile([1, T], BF16, tag="rows",
                                          name=f"ainvb_l{layer}_p{hp}_{r}")
                        nc.scalar.copy(rinvb[:, :], rinv[:, :])
                        ibp = ps512.tile([64, T], F32, tag="ps512",
                                         name=f"aib_l{layer}_p{hp}_{r}")
                        nc.tensor.matmul(ibp[:, :], ones_bf[0:1, 0:64],
                                         rinvb[:, :], start=True, stop=True)
                        ibs = sb.tile([64, T], F32, tag="ibs", bufs=2,
                                      name=f"aibs_l{layer}_p{hp}_{r}")
                        nc.scalar.copy(ibs[:, :], ibp[:, :])
                        nc.vector.tensor_mul(attnT[64 * r:64 * r + 64, hp, :],
                                             po[0:64, :], ibs[:, :])

                # ---- attn output projection + residual ----
                stats2 = LNStats(nc, sb, ps512, ones_bf, f"ln2_l{layer}")
                for i in range(DT):
                    wtile = sb.tile([128, DT, 128], BF16, tag="wqk", bufs=3,
                                    name=f"wo_l{layer}_i{i}")
                    nc.sync.dma_start(wtile[:, :, :], wo_d[layer, i, :, :, :])
                    pao = ps512.tile([128, T], F32, tag="ps512",
                                     name=f"pao_l{layer}_i{i}")
                    for j in range(DT):
                        nc.tensor.matmul(pao[:, :], wtile[:, j, :],
                                         attnT[:, j, :],
                                         start=(j == 0), stop=(j == DT - 1))
                    # x = x + attn_out + b_O
                    nc.vector.scalar_tensor_tensor(
                        xT[:, i, :], pao[:, :],
                        bias_sb[:, BC_BO + i:BC_BO + i + 1], xT[:, i, :],
                        ALU.add, ALU.add)
                    stats2.stat_tile(xT, i)

                # ---- LN2 ----
                h2 = sb.tile([128, DT, T], BF16, tag="h", bufs=2,
                             name=f"h2_l{layer}")
                _ln_finish(nc, rows, ps512, lntmp, stats2, xT, bias_sb,
                           BC_L2W, BC_L2B, h2, ones_f32, eps_sb)

                # ---- MLP in + gelu ----
                gT = sb.tile([128, MT, T], BF16, tag="gT",
                             name=f"gT_l{layer}")
                for j in range(MT):
                    wtile = sb.tile([128, DT, 128], BF16, tag="wqk", bufs=3,
                                    name=f"wi_l{layer}_j{j}")
                    nc.sync.dma_start(wtile[:, :, :], win_d[layer, j, :, :, :])
                    pg = ps512.tile([128, T], F32, tag="ps512",
                                    name=f"pg_l{layer}_j{j}")
                    for i in range(DT):
                        nc.tensor.matmul(pg[:, :], wtile[:, i, :], h2[:, i, :],
                                         start=(i == 0), stop=(i == DT - 1))
                    nc.scalar.activation(gT[:, j, :], pg[:, :],
                                         AF.Gelu_apprx_tanh,
                                         bias=bias_sb[:, BC_BIN + j:BC_BIN + j + 1])

                # ---- MLP out + residual ----
                stats_next = LNStats(nc, sb, ps512, ones_bf,
                                     f"ln1_l{layer + 1}")
                for i in range(DT):
                    wtile = sb.tile([128, MT, 128], BF16, tag="wout", bufs=2,
                                    name=f"wo2_l{layer}_i{i}")
                    nc.sync.dma_start(wtile[:, :, :], wout_d[layer, i, :, :, :])
                    pm = ps512.tile([128, T], F32, tag="ps512",
                                    name=f"pm_l{layer}_i{i}")
                    for j in range(MT):
                        nc.tensor.matmul(pm[:, :], wtile[:, j, :], gT[:, j, :],
                                         start=(j == 0), stop=(j == MT - 1))
                    nc.vector.scalar_tensor_tensor(
                        xT[:, i, :], pm[:, :],
                        bias_sb[:, BC_BOUT + i:BC_BOUT + i + 1], xT[:, i, :],
                        ALU.add, ALU.add)
                    stats_next.stat_tile(xT, i)

            # ---- final LN ----
            lnf_sb = sb.tile([128, 2 * DT], F32, tag="lnf")
            nc.sync.dma_start(lnf_sb[:, :], lnf_d.ap())
            xf = sb.tile([128, DT, T], BF16, tag="h", bufs=2, name="xf")
            _ln_finish(nc, rows, ps512, lntmp, stats_next, xT, lnf_sb, 0, DT,
                       xf, ones_f32, eps_sb)

            # ---- unembedding: logits[t, v] for all padded vocab slices ----
            for s in range(n_vslices):
                wu = sb.tile([128, DT, 512], BF16, tag="wu", bufs=3,
                             name=f"wu_s{s}")
                nc.sync.dma_start(wu[:, :, :], wu_d[s, :, :, :])
                for tt in range(4):
                    pu = ps512.tile([128, 512], F32, tag="ps512",
                                    name=f"pu_s{s}_t{tt}")
                    for i in range(DT):
                        nc.tensor.matmul(pu[:, :],
                                         xf[:, i, 128 * tt:128 * tt + 128],
                                         wu[:, i, :],
                                         start=(i == 0), stop=(i == DT - 1))
                    ou = sb.tile([128, 512], BF16, tag="ou", bufs=2,
                                 name=f"ou_s{s}_t{tt}")
                    if tt % 2 == 0:
                        nc.vector.tensor_copy(ou[:, :], pu[:, :])
                    else:
                        nc.scalar.copy(ou[:, :], pu[:, :])
                    nc.sync.dma_start(out_d[tt, :, s, :], ou[:, :])

    nc.compile()
    _BUILD_CACHE[key] = nc
    return nc


def _to_bf16(x):
    return np.ascontiguousarray(x.astype(ml_dtypes.bfloat16))


def prep_in_maps(inputs, n_layers=L, n_vslices=VS):
    """Host-side sharding: returns list of 8 per-core input dicts."""
    f = lambda k: np.asarray(inputs[k], dtype=np.float32)
    tokens = np.asarray(inputs["tokens"])
    W_E, W_pos = f("W_E"), f("W_pos")
    x_full = W_E[tokens] + W_pos[None, :S, :]        # [4, 1024, 768] f32

    nl = n_layers
    # fused QKV weight, feature-major lhsT layout [L, 128, DT, 2304]
    wq = f("W_Q").transpose(0, 2, 1, 3).reshape(L, D, D)[:nl]
    wk = f("W_K").transpose(0, 2, 1, 3).reshape(L, D, D)[:nl]
    wv = f("W_V").transpose(0, 2, 1, 3).reshape(L, D, D)[:nl]
    wqkc = np.concatenate([wq, wk], axis=2)           # [nl, 768, 1536]
    wqk = _to_bf16(wqkc.reshape(nl, DT, 128, 12, 128).transpose(0, 3, 2, 1, 4))
    wvp = _to_bf16(wv.reshape(nl, DT, 128, 2, 384).transpose(0, 3, 2, 1, 4))

    wo = f("W_O").reshape(L, D, D)[:nl]               # rows e = h*64+eh
    wo = _to_bf16(wo.reshape(nl, DT, 128, DT, 128).transpose(0, 3, 2, 1, 4))

    win = f("W_in")[:nl]                              # [nl, 768, 3072]
    win = _to_bf16(win.reshape(nl, DT, 128, MT, 128).transpose(0, 3, 2, 1, 4))

    wout = f("W_out")[:nl]                            # [nl, 3072, 768]
    wout = _to_bf16(wout.reshape(nl, MT, 128, DT, 128).transpose(0, 3, 2, 1, 4))

    wu_pad = np.zeros((D, VPAD), np.float32)
    wu_pad[:, :VOCAB] = f("W_U")
    wu = _to_bf16(wu_pad.reshape(DT, 128, VS, 512).transpose(2, 1, 0, 3))
    wu = np.ascontiguousarray(wu[:n_vslices])

    def percol(x, n):  # [nl, n*128] -> [nl, 128, n]
        return x.reshape(nl, n, 128).transpose(0, 2, 1)

    biases = np.zeros((nl, 128, BCOLS), np.float32)
    bq = f("b_Q").reshape(L, D)[:nl]
    bk = f("b_K").reshape(L, D)[:nl]
    biases[:, :, BC_QKVB:BC_QKVB + 12] = percol(
        np.concatenate([bq * 0.125, bk], axis=1), 12)
    biases[:, :, BC_BO:BC_BO + DT] = percol(f("b_O")[:nl], DT)
    biases[:, :, BC_BIN:BC_BIN + MT] = percol(f("b_in")[:nl], MT)
    biases[:, :, BC_BOUT:BC_BOUT + DT] = percol(f("b_out")[:nl], DT)
    biases[:, :, BC_L1W:BC_L1W + DT] = percol(f("ln1_w")[:nl], DT)
    biases[:, :, BC_L1B:BC_L1B + DT] = percol(f("ln1_b")[:nl], DT)
    biases[:, :, BC_L2W:BC_L2W + DT] = percol(f("ln2_w")[:nl], DT)
    biases[:, :, BC_L2B:BC_L2B + DT] = percol(f("ln2_b")[:nl], DT)
    bv = f("b_V").reshape(L, D)[:nl]
    biases[:, :, BC_BV:BC_BV + D] = np.repeat(bv[:, None, :], 128, axis=1)

    lnf = np.zeros((128, 2 * DT), np.float32)
    lnf[:, 0:DT] = f("lnf_w").reshape(DT, 128).T
    lnf[:, DT:2 * DT] = f("lnf_b").reshape(DT, 128).T

    # per-parity causal mask: key(global)=128*kt+p  vs  query(global)=512*h+q
    kk = np.arange(128)[:, None, None]
    tt = np.arange(8)[None, :, None]
    qq = np.arange(T)[None, None, :]
    masks = []
    for h in range(2):
        m = (128 * tt + kk <= 512 * h + qq).astype(np.float32)
        masks.append(_to_bf16(m))

    in_maps = []
    for c in range(NCORES):
        b, h = c // 2, c % 2
        xh = x_full[b, T * h:T * (h + 1)]             # [512, 768]
        x0 = np.ascontiguousarray(
            xh.reshape(T, DT, 128).transpose(2, 1, 0)).astype(np.float32)
        in_maps.append({
            "x0": x0, "wqk": wqk, "wv": wvp, "wo": wo, "win": win,
            "wout": wout, "wu": wu, "biases": biases, "lnf": lnf,
            "mask": masks[h],
        })
    return in_maps


def assemble_output(results, inputs, n_vslices=VS):
    """results: list of 8 per-core out dicts -> full [4, 1024, VOCAB] f32."""
    vp = n_vslices * 512
    out = np.zeros((B, S, VOCAB), np.float32)
    for c in range(NCORES):
        b, h = c // 2, c % 2
        arr = np.asarray(results[c]["out"]).astype(np.float32)  # [4,128,vs,512]
        flat = arr.reshape(T, vp)[:, :min(vp, VOCAB)]
        out[b, T * h:T * h + T, :flat.shape[1]] = flat
    out += np.asarray(inputs["b_U"], dtype=np.float32)[None, None, :]
    return out


def install_trace_hook():
    """Register the axon NTFF profiling hook (missing from this image's
    antenv) so run_bass_kernel_spmd(trace=True) returns exec_time_ns."""
    import sys as _sys
    import types as _types
    import ctypes as _ctypes
    import contextlib as _contextlib
    if "antenv.axon_hooks" in _sys.modules:
        return

    def _make_hook():
        lib = _ctypes.CDLL("/opt/axon/libaxon_pjrt.so")
        if not hasattr(lib, "axon_start_nrt_profile"):
            return None
        lib.axon_start_nrt_profile.argtypes = [
            _ctypes.POINTER(_ctypes.c_int64), _ctypes.c_size_t]
        lib.axon_start_nrt_profile.restype = _ctypes.c_int64
        lib.axon_stop_nrt_profile.argtypes = [_ctypes.c_char_p]
        lib.axon_stop_nrt_profile.restype = _ctypes.c_int64

        @_contextlib.contextmanager
        def _hook(output_dir, device_ids):
            import jax
            jax.devices()
            if device_ids:
                ids = (_ctypes.c_int64 * len(device_ids))(*device_ids)
                rc = lib.axon_start_nrt_profile(ids, len(device_ids))
            else:
                rc = lib.axon_start_nrt_profile(None, 0)
            if rc != 0:
                raise RuntimeError(f"axon_start_nrt_profile rc={rc}")
            try:
                yield
            finally:
                lib.axon_stop_nrt_profile(str(output_dir).encode())
        return _hook

    mod = _types.ModuleType("antenv.axon_hooks")
    mod.get_axon_ntff_profile_hook = lambda: _make_hook()
    _sys.modules["antenv.axon_hooks"] = mod


def run(inputs, n_layers=L, n_vslices=VS, trace=False, tmpdir=None):
    """Build, run, and assemble. Returns (output, exec_time_ns)."""
    nc = build(n_layers, n_vslices)
    in_maps = prep_in_maps(inputs, n_layers, n_vslices)
    kwargs = {}
    if trace:
        install_trace_hook()
        tmpdir = tmpdir or "/tmp/bk_trace"
        import shutil
        shutil.rmtree(tmpdir, ignore_errors=True)
        os.makedirs(tmpdir, exist_ok=True)
        kwargs = dict(trace=True, tmpdir=tmpdir)
    res = bass_utils.run_bass_kernel_spmd(
        nc, in_maps, core_ids=list(range(NCORES)), **kwargs)
    out = assemble_output(res.results, inputs, n_vslices)
    return out, res.exec_time_ns


def kernel(**inputs):
    trace = bool(int(os.environ.get("BK_TRACE", "0")))
    out, t = run(inputs, trace=trace,
                 tmpdir=os.environ.get("BK_TRACE_DIR"))
    if trace:
        print(f"HW exec time: {t} ns")
    return out


# revision 16
# speedup vs baseline: 1.3324x; 1.0784x over previous
"""GPT-2 small forward pass on 8 TRN2 NeuronCores (Bass/Tile).

Sharding: 8 cores = 4 batch elements x 2 sequence halves (512 tokens each).
Each core runs the full 12-layer trunk on its 512 tokens with replicated
weights; the only cross-core traffic is a per-layer 2-core AllGather of K/V
within each batch pair. Attention uses a transposed-score layout (keys on
partitions, queries on free dim) so the softmax denominator falls out of a
ones-augmented V matmul; causal masking is a multiplicative {0,1} bf16 mask
passed as per-core input data. All matmuls bf16 with fp32 PSUM accumulation;
residual stream and layernorm statistics in fp32.

Host side: embedding gather (W_E[tokens] + W_pos), weight repacking/bf16
cast, final unshard + b_U add.

Self-contained: only numpy/ml_dtypes/concourse imports; all shapes hardcoded.
"""

import os
import numpy as np
import ml_dtypes

import concourse.bass as bass
import concourse.mybir as mybir
import concourse.tile as tile
from concourse import bacc
from concourse import bass_utils

F32 = mybir.dt.float32
BF16 = mybir.dt.bfloat16
AF = mybir.ActivationFunctionType
ALU = mybir.AluOpType

# model dims
B, S, D, H, DH, DM, L, VOCAB = 4, 1024, 768, 12, 64, 3072, 12, 50257
T = 512              # tokens per core
NCORES = 8
DT = D // 128        # 6  d-tiles
MT = DM // 128       # 24 m-tiles of d_mlp
VS = (VOCAB + 511) // 512   # 99 vocab slices
VPAD = VS * 512      # 50688
EPS = 1e-5
PAIRS = [[0, 1], [2, 3], [4, 5], [6, 7]]

# bias-pack columns inside the per-layer [128, 840] f32 tensor
BC_QKVB = 0     # 12 cols: Q then K feature-tile biases
BC_BO = 12      # 6
BC_BIN = 18     # 24
BC_BOUT = 42    # 6
BC_L1W = 48     # 6
BC_L1B = 54     # 6
BC_L2W = 60     # 6
BC_L2B = 66     # 6
BC_BV = 72      # 768 (host-replicated across partitions)
BCOLS = 840

_BUILD_CACHE = {}


class LNStats:
    """Incremental layernorm statistics: per-d-tile column sums of x and x^2
    accumulated into two PSUM rows. Emit stat_tile(i) right after xT[:, i, :]
    is finalized (inside the preceding residual-add loop) so the ones-matmuls
    interleave with that phase's main matmul stream instead of stalling PE."""

    def __init__(self, nc, sb, ps512, ones_bf, name):
        self.nc, self.sb, self.ones_bf = nc, sb, ones_bf
        self.s1 = ps512.tile([1, T], F32, tag="ps512", name=f"{name}_s1")
        self.s2 = ps512.tile([1, T], F32, tag="ps512", name=f"{name}_s2")
        self.name = name

    def stat_tile(self, x_sb, i):
        nc, sb = self.nc, self.sb
        xb = sb.tile([128, T], BF16, tag="lnxb", bufs=3,
                     name=f"{self.name}_xb{i}")
        nc.scalar.copy(xb[:, :], x_sb[:, i, :])
        xsq = sb.tile([128, T], BF16, tag="lnxsq", bufs=3,
                      name=f"{self.name}_xsq{i}")
        nc.vector.tensor_mul(xsq[:, :], xb[:, :], xb[:, :])
        nc.tensor.matmul(self.s1[:, :], self.ones_bf[:, 0:1], xb[:, :],
                         start=(i == 0), stop=(i == DT - 1))
        nc.tensor.matmul(self.s2[:, :], self.ones_bf[:, 0:1], xsq[:, :],
                         start=(i == 0), stop=(i == DT - 1))


def _ln_finish(nc, rows, ps512, lntmp, stats, x_sb, bias_sb, wcol, bcol,
               out_bf, ones_f32, eps_sb, n_feat=D):
    """Finish LN from accumulated stats: rows chain, broadcast, normalize.
    out_bf[:, i, :] lands per-slice so downstream matmuls on slice i can
    start before slice i+1 exists."""
    s1, s2, name = stats.s1, stats.s2, stats.name
    row_mean = rows.tile([1, T], F32, tag="rows", name=f"{name}_mean")
    nc.vector.tensor_scalar_mul(row_mean[:, :], s1[:, :], 1.0 / n_feat)
    row_mean_bf = rows.tile([1, T], BF16, tag="rows", name=f"{name}_meanb")
    nc.scalar.copy(row_mean_bf[:, :], row_mean[:, :])
    mb = ps512.tile([128, T], F32, tag="ps512", name=f"{name}_mb")
    nc.tensor.matmul(mb[:, :], stats.ones_bf[0:1, 0:128], row_mean_bf[:, :],
                     start=True, stop=True)
    row_m2 = rows.tile([1, T], F32, tag="rows", name=f"{name}_m2")
    nc.vector.tensor_mul(row_m2[:, :], s1[:, :], row_mean[:, :])
    row_var = rows.tile([1, T], F32, tag="rows", name=f"{name}_var")
    nc.vector.tensor_sub(row_var[:, :], s2[:, :], row_m2[:, :])
    # unbiased variance (ddof=1): var = (sum_x2 - sum_x*mean) / (n-1)
    # invstd = 1/sqrt(var/(n-1) + eps) in one ScalarE LUT op
    row_inv_bf = rows.tile([1, T], BF16, tag="rows", name=f"{name}_invb")
    nc.scalar.activation(row_inv_bf[:, :], row_var[:, :],
                         AF.Abs_reciprocal_sqrt,
                         bias=eps_sb[0:1, 0:1], scale=1.0 / (n_feat - 1))
    ib = ps512.tile([128, T], F32, tag="ps512", name=f"{name}_ib")
    nc.tensor.matmul(ib[:, :], stats.ones_bf[0:1, 0:128], row_inv_bf[:, :],
                     start=True, stop=True)

    for i in range(DT):
        t0 = lntmp.tile([128, T], F32, tag="lntmp", name=f"{name}_t{i}")
        nc.vector.tensor_sub(t0[:, :], x_sb[:, i, :], mb[:, :])
        u0 = lntmp.tile([128, T], F32, tag="lntmp", name=f"{name}_u{i}")
        nc.vector.tensor_mul(u0[:, :], t0[:, :], ib[:, :])
        nc.scalar.activation(out_bf[:, i, :], u0[:, :], AF.Identity,
                             bias=bias_sb[:, bcol + i:bcol + i + 1],
                             scale=bias_sb[:, wcol + i:wcol + i + 1])


def build(n_layers=L, n_vslices=VS):
    """Build + compile the SPMD kernel. Returns the Bacc object."""
    key = (n_layers, n_vslices)
    if key in _BUILD_CACHE:
        return _BUILD_CACHE[key]

    nc = bacc.Bacc("TRN2", target_bir_lowering=False, debug=False,
                   enable_asserts=False, num_devices=NCORES)

    # ---- kernel I/O (per-core shards; all cores same shapes) ----
    x0_d = nc.dram_tensor("x0", [128, DT, T], F32, kind="ExternalInput")
    wqk_d = nc.dram_tensor("wqk", [n_layers, 12, 128, DT, 128], BF16,
                           kind="ExternalInput")
    wv_d = nc.dram_tensor("wv", [n_layers, 2, 128, DT, 384], BF16,
                          kind="ExternalInput")
    wo_d = nc.dram_tensor("wo", [n_layers, DT, 128, DT, 128], BF16,
                          kind="ExternalInput")
    win_d = nc.dram_tensor("win", [n_layers, MT, 128, DT, 128], BF16,
                           kind="ExternalInput")
    wout_d = nc.dram_tensor("wout", [n_layers, DT, 128, MT, 128], BF16,
                            kind="ExternalInput")
    wu_d = nc.dram_tensor("wu", [n_vslices, 128, DT, 512], BF16,
                          kind="ExternalInput")
    bias_d = nc.dram_tensor("biases", [n_layers, 128, BCOLS], F32,
                            kind="ExternalInput")
    lnf_d = nc.dram_tensor("lnf", [128, 2 * DT], F32, kind="ExternalInput")
    mask_d = nc.dram_tensor("mask", [128, 8, T], BF16, kind="ExternalInput")
    out_d = nc.dram_tensor("out", [4, 128, n_vslices, 512], BF16,
                           kind="ExternalOutput")

    with tile.TileContext(nc) as tc:
        with tc.tile_pool(name="sb", bufs=1) as sb, \
             tc.tile_pool(name="rows", bufs=6) as rows, \
             tc.tile_pool(name="lntmp", bufs=2) as lntmp, \
             tc.tile_pool(name="ps512", bufs=5, space="PSUM") as ps512, \
             tc.tile_pool(name="pso", bufs=3, space="PSUM") as pso, \
             tc.tile_pool(name="dram", bufs=2, space="DRAM") as dram:

            # ---- persistent tiles ----
            ones_f32 = sb.tile([128, 128], F32, tag="ones_f32")
            nc.vector.memset(ones_f32[:, :], 1.0)
            ones_bf = sb.tile([128, 128], BF16, tag="ones_bf")
            nc.vector.memset(ones_bf[:, :], 1.0)
            eps_sb = sb.tile([128, 1], F32, tag="eps")
            nc.vector.memset(eps_sb[:, :], EPS)

            xT = sb.tile([128, DT, T], F32, tag="xT")
            nc.sync.dma_start(xT[:, :, :], x0_d.ap())
            stats_next = LNStats(nc, sb, ps512, ones_bf, "ln1_l0")
            for i in range(DT):
                stats_next.stat_tile(xT, i)

            mask_sb = sb.tile([128, 8, T], BF16, tag="mask")
            nc.sync.dma_start(mask_sb[:, :, :], mask_d.ap())

            # K/V of both sequence halves, in global token order
            ktall = sb.tile([128, DT, 2 * T], BF16, tag="ktall")
            vall = sb.tile([128, 8, H, 65], BF16, tag="vall")
            v_own = sb.tile([128, 4, H, 65], BF16, tag="vown")
            nc.vector.memset(v_own[:, :, :, 64:65], 1.0)  # denominator ones col

            for layer in range(n_layers):
                bias_sb = sb.tile([128, BCOLS], F32, tag="bias", bufs=2,
                                  name=f"bias_l{layer}")
                nc.sync.dma_start(bias_sb[:, :], bias_d[layer, :, :])

                # ---- LN1 (stats pre-accumulated in the previous phase) ----
                h_bf = sb.tile([128, DT, T], BF16, tag="h", bufs=2,
                               name=f"h1_l{layer}")
                _ln_finish(nc, rows, ps512, lntmp, stats_next, xT, bias_sb,
                           BC_L1W, BC_L1B, h_bf, ones_f32, eps_sb)

                # ---- K projection (feature-major KT) ----
                kt_own = sb.tile([128, DT, T], BF16, tag="kta", bufs=2,
                                 name=f"ktown_l{layer}")
                for m in range(6, 12):
                    wtile = sb.tile([128, DT, 128], BF16, tag="wqk", bufs=3,
                                    name=f"wk_l{layer}_m{m}")
                    nc.sync.dma_start(wtile[:, :, :], wqk_d[layer, m, :, :, :])
                    psq = ps512.tile([128, T], F32, tag="ps512",
                                     name=f"psk_l{layer}_m{m}")
                    for i in range(DT):
                        nc.tensor.matmul(psq[:, :], wtile[:, i, :], h_bf[:, i, :],
                                         start=(i == 0), stop=(i == DT - 1))
                    nc.scalar.activation(
                        kt_own[:, m - 6, :], psq[:, :], AF.Identity,
                        bias=bias_sb[:, BC_QKVB + m:BC_QKVB + m + 1])

                # ---- V projection (token-major, lhsT = h tiles) ----
                for half in range(2):
                    wv = sb.tile([128, DT, 384], BF16, tag="wv", bufs=2,
                                 name=f"wv_l{layer}_{half}")
                    nc.sync.dma_start(wv[:, :, :], wv_d[layer, half, :, :, :])
                    for tt in range(4):
                        psv = ps512.tile([128, 384], F32, tag="ps512",
                                         name=f"psv_l{layer}_{half}_{tt}")
                        for i in range(DT):
                            nc.tensor.matmul(psv[:, :],
                                             h_bf[:, i, 128 * tt:128 * tt + 128],
                                             wv[:, i, :],
                                             start=(i == 0), stop=(i == DT - 1))
                        nc.vector.tensor_add(
                            v_own[:, tt, 6 * half:6 * half + 6, 0:64], psv[:, :],
                            bias_sb[:, BC_BV + 384 * half:BC_BV + 384 * (half + 1)])

                # ---- pair exchange of K/V ----
                KTN = DT * T
                VN = 4 * H * 65
                bounce_in = dram.tile([128, KTN + VN], BF16, tag="cin",
                                      name=f"cin_l{layer}")
                bounce_out = dram.tile([256, KTN + VN], BF16, tag="cout",
                                       name=f"cout_l{layer}")
                nc.sync.dma_start(bounce_in[:, 0:KTN], kt_own[:, :, :])
                nc.sync.dma_start(bounce_in[:, KTN:KTN + VN],
                                  v_own[:, :, :, :])
                nc.gpsimd.collective_compute(
                    "AllGather", ALU.bypass, replica_groups=PAIRS,
                    ins=[bounce_in[:, :].opt()], outs=[bounce_out[:, :].opt()])
                for c in range(2):
                    nc.sync.dma_start(
                        ktall[:, :, T * c:T * (c + 1)],
                        bounce_out[128 * c:128 * c + 128, 0:KTN].rearrange(
                            "p (i t) -> p i t", i=DT))
                    nc.sync.dma_start(
                        vall[:, 4 * c:4 * (c + 1), :, :],
                        bounce_out[128 * c:128 * c + 128, KTN:KTN + VN])

                # ---- Q projection (overlaps the collective) ----
                qt = sb.tile([128, DT, T], BF16, tag="qt", bufs=1,
                             name=f"qt_l{layer}")
                for m in range(6):
                    wtile = sb.tile([128, DT, 128], BF16, tag="wqk", bufs=3,
                                    name=f"wq_l{layer}_m{m}")
                    nc.sync.dma_start(wtile[:, :, :], wqk_d[layer, m, :, :, :])
                    psq = ps512.tile([128, T], F32, tag="ps512",
                                     name=f"psq_l{layer}_m{m}")
                    for i in range(DT):
                        nc.tensor.matmul(psq[:, :], wtile[:, i, :], h_bf[:, i, :],
                                         start=(i == 0), stop=(i == DT - 1))
                    # (Q + b) / sqrt(DH); host stores b_Q / 8
                    nc.scalar.activation(
                        qt[:, m, :], psq[:, :], AF.Identity,
                        bias=bias_sb[:, BC_QKVB + m:BC_QKVB + m + 1],
                        scale=0.125)

                # ---- attention, head-pair software pipeline ----
                attnT = sb.tile([128, DT, T], BF16, tag="kta", bufs=2,
                                name=f"attnT_l{layer}")
                # Heads 2hp (rows 0:64) and 2hp+1 (rows 64:128) issue adjacent
                # score matmuls that run concurrently in distinct row groups.
                # PV matmuls for k-tile kt-1 interleave with scores of kt.
                for hp in range(H // 2):
                    ha, hb = 2 * hp, 2 * hp + 1
                    po_a = pso.tile([65, T], F32, tag="pso",
                                    name=f"po_l{layer}_h{ha}")
                    po_b = pso.tile([65, T], F32, tag="pso",
                                    name=f"po_l{layer}_h{hb}")
                    esm_prev = None
                    for kt in range(8):
                        ks = slice(128 * kt, 128 * kt + 128)
                        sps_a = ps512.tile([128, T], F32, tag="ps512",
                                           name=f"s_l{layer}_h{ha}_k{kt}")
                        nc.tensor.matmul(sps_a[:, :], ktall[0:64, hp, ks],
                                         qt[0:64, hp, :],
                                         start=True, stop=True)
                        sps_b = ps512.tile([128, T], F32, tag="ps512",
                                           name=f"s_l{layer}_h{hb}_k{kt}")
                        nc.tensor.matmul(sps_b[:, :], ktall[64:128, hp, ks],
                                         qt[64:128, hp, :],
                                         start=True, stop=True)
                        if esm_prev is not None:
                            pk = kt - 1
                            nc.tensor.matmul(po_a[:, :], vall[:, pk, ha, :],
                                             esm_prev[0][:, :],
                                             start=(pk == 0), stop=False)
                            nc.tensor.matmul(po_b[:, :], vall[:, pk, hb, :],
                                             esm_prev[1][:, :],
                                             start=(pk == 0), stop=False)
                        pair = []
                        for hd, sps in ((ha, sps_a), (hb, sps_b)):
                            es = sb.tile([128, T], BF16, tag="es", bufs=3,
                                         name=f"es_l{layer}_h{hd}_k{kt}")
                            nc.scalar.activation(es[:, :], sps[:, :], AF.Exp)
                            esm = sb.tile([128, T], BF16, tag="esm", bufs=6,
                                          name=f"esm_l{layer}_h{hd}_k{kt}")
                            nc.vector.tensor_mul(esm[:, :], es[:, :],
                                                 mask_sb[:, kt, :])
                            pair.append(esm)
                        esm_prev = pair
                    nc.tensor.matmul(po_a[:, :], vall[:, 7, ha, :],
                                     esm_prev[0][:, :], start=False, stop=True)
                    nc.tensor.matmul(po_b[:, :], vall[:, 7, hb, :],
                                     esm_prev[1][:, :], start=False, stop=True)
                    for r, po in ((0, po_a), (1, po_b)):
                        rr = rows.tile([1, T], F32, tag="rows",
                                       name=f"arsq_l{layer}_p{hp}_{r}")
                        nc.scalar.activation(rr[:, :], po[64:65, :],
                                             AF.Abs_reciprocal_sqrt)
                        rinvb = rows.tile([1, T], BF16, tag="rows",
                                          name=f"ainvb_l{layer}_p{hp}_{r}")
                        nc.vector.tensor_mul(rinvb[:, :], rr[:, :], rr[:, :])
                        ibp = ps512.tile([64, T], F32, tag="ps512",
                                         name=f"aib_l{layer}_p{hp}_{r}")
                        nc.tensor.matmul(ibp[:, :], ones_bf[0:1, 0:64],
                                         rinvb[:, :], start=True, stop=True)
                        ibs = sb.tile([64, T], F32, tag="ibs", bufs=2,
                                      name=f"aibs_l{layer}_p{hp}_{r}")
                        nc.scalar.copy(ibs[:, :], ibp[:, :])
                        nc.vector.tensor_mul(attnT[64 * r:64 * r + 64, hp, :],
                                             po[0:64, :], ibs[:, :])

                # ---- attn output projection + residual ----
                stats2 = LNStats(nc, sb, ps512, ones_bf, f"ln2_l{layer}")
                for i in range(DT):
                    wtile = sb.tile([128, DT, 128], BF16, tag="wqk", bufs=3,
                                    name=f"wo_l{layer}_i{i}")
                    nc.sync.dma_start(wtile[:, :, :], wo_d[layer, i, :, :, :])
                    pao = ps512.tile([128, T], F32, tag="ps512",
                                     name=f"pao_l{layer}_i{i}")
                    for j in range(DT):
                        nc.tensor.matmul(pao[:, :], wtile[:, j, :],
                                         attnT[:, j, :],
                                         start=(j == 0), stop=(j == DT - 1))
                    # x = x + attn_out + b_O
                    nc.vector.scalar_tensor_tensor(
                        xT[:, i, :], pao[:, :],
                        bias_sb[:, BC_BO + i:BC_BO + i + 1], xT[:, i, :],
                        ALU.add, ALU.add)
                    stats2.stat_tile(xT, i)

                # ---- LN2 ----
                h2 = sb.tile([128, DT, T], BF16, tag="h", bufs=2,
                             name=f"h2_l{layer}")
                _ln_finish(nc, rows, ps512, lntmp, stats2, xT, bias_sb,
                           BC_L2W, BC_L2B, h2, ones_f32, eps_sb)

                # ---- MLP in + gelu ----
                gT = sb.tile([128, MT, T], BF16, tag="gT",
                             name=f"gT_l{layer}")
                for j in range(MT):
                    wtile = sb.tile([128, DT, 128], BF16, tag="wqk", bufs=3,
                                    name=f"wi_l{layer}_j{j}")
                    nc.sync.dma_start(wtile[:, :, :], win_d[layer, j, :, :, :])
                    pg = ps512.tile([128, T], F32, tag="ps512",
                                    name=f"pg_l{layer}_j{j}")
                    for i in range(DT):
                        nc.tensor.matmul(pg[:, :], wtile[:, i, :], h2[:, i, :],
                                         start=(i == 0), stop=(i == DT - 1))
                    nc.scalar.activation(gT[:, j, :], pg[:, :],
                                         AF.Gelu_apprx_tanh,
                                         bias=bias_sb[:, BC_BIN + j:BC_BIN + j + 1])

                # ---- MLP out + residual ----
                stats_next = LNStats(nc, sb, ps512, ones_bf,
                                     f"ln1_l{layer + 1}")
                for i in range(DT):
                    wtile = sb.tile([128, MT, 128], BF16, tag="wout", bufs=2,
                                    name=f"wo2_l{layer}_i{i}")
                    nc.sync.dma_start(wtile[:, :, :], wout_d[layer, i, :, :, :])
                    pm = ps512.tile([128, T], F32, tag="ps512",
                                    name=f"pm_l{layer}_i{i}")
                    for j in range(MT):
                        nc.tensor.matmul(pm[:, :], wtile[:, j, :], gT[:, j, :],
                                         start=(j == 0), stop=(j == MT - 1))
                    nc.vector.scalar_tensor_tensor(
                        xT[:, i, :], pm[:, :],
                        bias_sb[:, BC_BOUT + i:BC_BOUT + i + 1], xT[:, i, :],
                        ALU.add, ALU.add)
                    stats_next.stat_tile(xT, i)

            # ---- final LN ----
            lnf_sb = sb.tile([128, 2 * DT], F32, tag="lnf")
            nc.sync.dma_start(lnf_sb[:, :], lnf_d.ap())
            xf = sb.tile([128, DT, T], BF16, tag="h", bufs=2, name="xf")
            _ln_finish(nc, rows, ps512, lntmp, stats_next, xT, lnf_sb, 0, DT,
                       xf, ones_f32, eps_sb)

            # ---- unembedding: logits[t, v] for all padded vocab slices ----
            for s in range(n_vslices):
                wu = sb.tile([128, DT, 512], BF16, tag="wu", bufs=3,
                             name=f"wu_s{s}")
                nc.sync.dma_start(wu[:, :, :], wu_d[s, :, :, :])
                for tt in range(4):
                    pu = ps512.tile([128, 512], F32, tag="ps512",
                                    name=f"pu_s{s}_t{tt}")
                    for i in range(DT):
                        nc.tensor.matmul(pu[:, :],
                                         xf[:, i, 128 * tt:128 * tt + 128],
                                         wu[:, i, :],
                                         start=(i == 0), stop=(i == DT - 1))
                    ou = sb.tile([128, 512], BF16, tag="ou", bufs=2,
                                 name=f"ou_s{s}_t{tt}")
                    if tt % 2 == 0:
                        nc.vector.tensor_copy(ou[:, :], pu[:, :])
                    else:
                        nc.scalar.copy(ou[:, :], pu[:, :])
                    nc.sync.dma_start(out_d[tt, :, s, :], ou[:, :])

    nc.compile()
    _BUILD_CACHE[key] = nc
    return nc


def _to_bf16(x):
    return np.ascontiguousarray(x.astype(ml_dtypes.bfloat16))


def prep_in_maps(inputs, n_layers=L, n_vslices=VS):
    """Host-side sharding: returns list of 8 per-core input dicts."""
    f = lambda k: np.asarray(inputs[k], dtype=np.float32)
    tokens = np.asarray(inputs["tokens"])
    W_E, W_pos = f("W_E"), f("W_pos")
    x_full = W_E[tokens] + W_pos[None, :S, :]        # [4, 1024, 768] f32

    nl = n_layers
    # fused QKV weight, feature-major lhsT layout [L, 128, DT, 2304]
    wq = f("W_Q").transpose(0, 2, 1, 3).reshape(L, D, D)[:nl]
    wk = f("W_K").transpose(0, 2, 1, 3).reshape(L, D, D)[:nl]
    wv = f("W_V").transpose(0, 2, 1, 3).reshape(L, D, D)[:nl]
    wqkc = np.concatenate([wq, wk], axis=2)           # [nl, 768, 1536]
    wqk = _to_bf16(wqkc.reshape(nl, DT, 128, 12, 128).transpose(0, 3, 2, 1, 4))
    wvp = _to_bf16(wv.reshape(nl, DT, 128, 2, 384).transpose(0, 3, 2, 1, 4))

    wo = f("W_O").reshape(L, D, D)[:nl]               # rows e = h*64+eh
    wo = _to_bf16(wo.reshape(nl, DT, 128, DT, 128).transpose(0, 3, 2, 1, 4))

    win = f("W_in")[:nl]                              # [nl, 768, 3072]
    win = _to_bf16(win.reshape(nl, DT, 128, MT, 128).transpose(0, 3, 2, 1, 4))

    wout = f("W_out")[:nl]                            # [nl, 3072, 768]
    wout = _to_bf16(wout.reshape(nl, MT, 128, DT, 128).transpose(0, 3, 2, 1, 4))

    wu_pad = np.zeros((D, VPAD), np.float32)
    wu_pad[:, :VOCAB] = f("W_U")
    wu = _to_bf16(wu_pad.reshape(DT, 128, VS, 512).transpose(2, 1, 0, 3))
    wu = np.ascontiguousarray(wu[:n_vslices])

    def percol(x, n):  # [nl, n*128] -> [nl, 128, n]
        return x.reshape(nl, n, 128).transpose(0, 2, 1)

    biases = np.zeros((nl, 128, BCOLS), np.float32)
    bq = f("b_Q").reshape(L, D)[:nl]
    bk = f("b_K").reshape(L, D)[:nl]
    biases[:, :, BC_QKVB:BC_QKVB + 12] = percol(
        np.concatenate([bq * 0.125, bk], axis=1), 12)
    biases[:, :, BC_BO:BC_BO + DT] = percol(f("b_O")[:nl], DT)
    biases[:, :, BC_BIN:BC_BIN + MT] = percol(f("b_in")[:nl], MT)
    biases[:, :, BC_BOUT:BC_BOUT + DT] = percol(f("b_out")[:nl], DT)
    biases[:, :, BC_L1W:BC_L1W + DT] = percol(f("ln1_w")[:nl], DT)
    biases[:, :, BC_L1B:BC_L1B + DT] = percol(f("ln1_b")[:nl], DT)
    biases[:, :, BC_L2W:BC_L2W + DT] = percol(f("ln2_w")[:nl], DT)
    biases[:, :, BC_L2B:BC_L2B + DT] = percol(f("ln2_b")[:nl], DT)
    bv = f("b_V").reshape(L, D)[:nl]
    biases[:, :, BC_BV:BC_BV + D] = np.repeat(bv[:, None, :], 128, axis=1)

    lnf = np.zeros((128, 2 * DT), np.float32)
    lnf[:, 0:DT] = f("lnf_w").reshape(DT, 128).T
    lnf[:, DT:2 * DT] = f("lnf_b").reshape(DT, 128).T

    # per-parity causal mask: key(global)=128*kt+p  vs  query(global)=512*h+q
    kk = np.arange(128)[:, None, None]
    tt = np.arange(8)[None, :, None]
    qq = np.arange(T)[None, None, :]
    masks = []
    for h in range(2):
        m = (128 * tt + kk <= 512 * h + qq).astype(np.float32)
        masks.append(_to_bf16(m))

    in_maps = []
    for c in range(NCORES):
        b, h = c // 2, c % 2
        xh = x_full[b, T * h:T * (h + 1)]             # [512, 768]
        x0 = np.ascontiguousarray(
            xh.reshape(T, DT, 128).transpose(2, 1, 0)).astype(np.float32)
        in_maps.append({
            "x0": x0, "wqk": wqk, "wv": wvp, "wo": wo, "win": win,
            "wout": wout, "wu": wu, "biases": biases, "lnf": lnf,
            "mask": masks[h],
        })
    return in_maps


def assemble_output(results, inputs, n_vslices=VS):
    """results: list of 8 per-core out dicts -> full [4, 1024, VOCAB] f32."""
    vp = n_vslices * 512
    out = np.zeros((B, S, VOCAB), np.float32)
    for c in range(NCORES):
        b, h = c // 2, c % 2
        arr = np.asarray(results[c]["out"]).astype(np.float32)  # [4,128,vs,512]
        flat = arr.reshape(T, vp)[:, :min(vp, VOCAB)]
        out[b, T * h:T * h + T, :flat.shape[1]] = flat
    out += np.asarray(inputs["b_U"], dtype=np.float32)[None, None, :]
    return out


def install_trace_hook():
    """Register the axon NTFF profiling hook (missing from this image's
    antenv) so run_bass_kernel_spmd(trace=True) returns exec_time_ns."""
    import sys as _sys
    import types as _types
    import ctypes as _ctypes
    import contextlib as _contextlib
    if "antenv.axon_hooks" in _sys.modules:
        return

    def _make_hook():
        lib = _ctypes.CDLL("/opt/axon/libaxon_pjrt.so")
        if not hasattr(lib, "axon_start_nrt_profile"):
            return None
        lib.axon_start_nrt_profile.argtypes = [
            _ctypes.POINTER(_ctypes.c_int64), _ctypes.c_size_t]
        lib.axon_start_nrt_profile.restype = _ctypes.c_int64
        lib.axon_stop_nrt_profile.argtypes = [_ctypes.c_char_p]
        lib.axon_stop_nrt_profile.restype = _ctypes.c_int64

        @_contextlib.contextmanager
        def _hook(output_dir, device_ids):
            import jax
            jax.devices()
            if device_ids:
                ids = (_ctypes.c_int64 * len(device_ids))(*device_ids)
                rc = lib.axon_start_nrt_profile(ids, len(device_ids))
            else:
                rc = lib.axon_start_nrt_profile(None, 0)
            if rc != 0:
                raise RuntimeError(f"axon_start_nrt_profile rc={rc}")
            try:
                yield
            finally:
                lib.axon_stop_nrt_profile(str(output_dir).encode())
        return _hook

    mod = _types.ModuleType("antenv.axon_hooks")
    mod.get_axon_ntff_profile_hook = lambda: _make_hook()
    _sys.modules["antenv.axon_hooks"] = mod


def run(inputs, n_layers=L, n_vslices=VS, trace=False, tmpdir=None):
    """Build, run, and assemble. Returns (output, exec_time_ns)."""
    nc = build(n_layers, n_vslices)
    in_maps = prep_in_maps(inputs, n_layers, n_vslices)
    kwargs = {}
    if trace:
        install_trace_hook()
        tmpdir = tmpdir or "/tmp/bk_trace"
        import shutil
        shutil.rmtree(tmpdir, ignore_errors=True)
        os.makedirs(tmpdir, exist_ok=True)
        kwargs = dict(trace=True, tmpdir=tmpdir)
    res = bass_utils.run_bass_kernel_spmd(
        nc, in_maps, core_ids=list(range(NCORES)), **kwargs)
    out = assemble_output(res.results, inputs, n_vslices)
    return out, res.exec_time_ns


def kernel(**inputs):
    trace = bool(int(os.environ.get("BK_TRACE", "0")))
    out, t = run(inputs, trace=trace,
                 tmpdir=os.environ.get("BK_TRACE_DIR"))
    if trace:
        print(f"HW exec time: {t} ns")
    return out
